# revision 1
# baseline (speedup 1.0000x reference)
"""FAGCN (4-layer FAConv + lin1/lin2 + log_softmax) on 8 Trainium2 cores.

Strategy (graph/data parallel, per the sharding hint):
- Nodes sharded across 8 cores (6250 each). Within a core, nodes are
  degree-sorted and packed into 49 tiles of 128 (CSR layout: partition p of
  tile t = that tile's p-th node; its incoming edges occupy slot columns).
- Per layer: h-table rows [h(64) | al | pad] (128 f32 = 512B) are all-gathered
  to every core; h[src]+al[src] per edge is fetched with dma_gather.
  dma_gather indices are int16 (<32768) so the 50176-row table is covered by
  two windows: A=[0,32768) and B=[RF-32768,RF). Each node's edge list is
  split between the windows; rows in the overlap [RF-32768,32768) can use
  either window and are assigned to balance the split (no negative/skip
  indices needed -> single_packet=False large gathers are safe).
- coeff = tanh(al_src + ar_dst) * norm with ar as a per-partition ACT bias,
  messages scaled on DVE (bf16) and segment-summed via identity matmuls into
  PSUM. h_new = segsum + EPS*raw. Final: logits + log_softmax on-device.
"""
import numpy as np
from dataclasses import dataclass

import concourse.bass as bass
import concourse.bacc as bacc
import concourse.tile as tile
import concourse.mybir as mybir
from concourse import bass_utils
from concourse.masks import make_identity

F32 = mybir.dt.float32
BF16 = mybir.dt.bfloat16
I16 = mybir.dt.int16
AF = mybir.ActivationFunctionType
OP = mybir.AluOpType


@dataclass
class Cfg:
    N: int = 50000
    E: int = 800000
    F: int = 512
    H: int = 64
    C: int = 40
    L: int = 4
    EPS: float = 0.2
    M: int = 8           # cores
    CHUNK_COLS: int = 32
    WINDOW: int = 32768  # dma_gather int16 index limit

    @property
    def NSH(self):
        return self.N // self.M

    @property
    def TPC(self):
        return (self.NSH + 127) // 128

    @property
    def NSHP(self):
        return self.TPC * 128

    @property
    def RF(self):
        return self.NSHP * self.M

    @property
    def two_windows(self):
        return self.RF > self.WINDOW


def host_prep(cfg: Cfg, x, edge_index, W1, b1, W2, b2, att_l, att_r):
    """Shard + permute + build balanced window-split gather arrays."""
    N, M, NSH, NSHP, TPC = cfg.N, cfg.M, cfg.NSH, cfg.NSHP, cfg.TPC
    src = np.asarray(edge_index[0], dtype=np.int64)
    dst = np.asarray(edge_index[1], dtype=np.int64)
    loop = np.arange(N, dtype=np.int64)
    rows = np.concatenate([src, loop])
    cols = np.concatenate([dst, loop])
    deg = np.bincount(cols, minlength=N).astype(np.float32)
    dinv = (1.0 / np.sqrt(deg)).astype(np.float32)
    norm_e = (dinv[rows] * dinv[cols]).astype(np.float32)

    core_of = cols // NSH
    orders, inv_orders = [], []
    for k in range(M):
        degl = np.bincount(cols[core_of == k] - k * NSH, minlength=NSH)
        order = np.argsort(-degl, kind="stable")
        inv = np.empty(NSH, dtype=np.int64)
        inv[order] = np.arange(NSH)
        orders.append(order)
        inv_orders.append(inv)
    grow_map = np.empty(N, dtype=np.int64)
    for k in range(M):
        grow_map[k * NSH:(k + 1) * NSH] = k * NSHP + inv_orders[k]

    B_BASE = cfg.RF - cfg.WINDOW  # window B covers [B_BASE, RF)

    # pass 1: per-core per-node A/B counts -> shared CA/CB per tile
    percore = []
    CA = np.zeros(TPC, dtype=np.int64)
    CB = np.zeros(TPC, dtype=np.int64)
    for k in range(M):
        m = core_of == k
        es, en = rows[m], norm_e[m]
        rk = inv_orders[k][cols[m] - k * NSH]
        grow = grow_map[es]
        if cfg.two_windows:
            cls = np.where(grow >= cfg.WINDOW, 2,
                           np.where(grow >= B_BASE, 1, 0)).astype(np.int8)
        else:
            cls = np.zeros(len(es), np.int8)
        n0 = np.bincount(rk[cls == 0], minlength=NSHP)
        n1 = np.bincount(rk[cls == 1], minlength=NSHP)
        n2 = np.bincount(rk[cls == 2], minlength=NSHP)
        d = n0 + n1 + n2
        tgt = (d + 1) // 2
        nlo = np.minimum(np.maximum(n0, tgt), n0 + n1)
        if not cfg.two_windows:
            nlo = d
        nhi = d - nlo
        for t in range(TPC):
            s = slice(t * 128, (t + 1) * 128)
            CA[t] = max(CA[t], nlo[s].max(), 1)
            CB[t] = max(CB[t], nhi[s].max())
        percore.append((es, rk, en, grow, cls, nlo))
    offA = np.zeros(TPC + 1, dtype=np.int64)
    np.cumsum(CA, out=offA[1:])
    offB = np.zeros(TPC + 1, dtype=np.int64)
    np.cumsum(CB, out=offB[1:])
    TA, TB = int(offA[-1]), int(offB[-1])

    def wrap16(lst16):
        a = lst16.reshape(-1, 16).T.copy()
        return np.tile(a, (8, 1)).astype(np.int16)

    in_maps = []
    for k in range(M):
        es, rk, en, grow, cls, nlo = percore[k]
        # order edges per node by class (lo-fixed, flex, hi-fixed)
        o = np.lexsort((cls, rk))
        rk, en, grow, cls = rk[o], en[o], grow[o], cls[o]
        dl = np.bincount(rk, minlength=NSHP)
        run0 = np.repeat(np.cumsum(np.concatenate([[0], dl]))[:-1], dl)
        j = np.arange(len(rk)) - run0           # index within node's list
        is_lo = j < nlo[rk]
        p_all = rk % 128
        t_all = rk // 128
        colA = offA[t_all] + j                  # for lo edges
        colB = offB[t_all] + (j - nlo[rk])      # for hi edges
        posA = colA[is_lo] * 128 + p_all[is_lo]
        posB = colB[~is_lo] * 128 + p_all[~is_lo]

        idxA = np.zeros(TA * 128, dtype=np.int64)
        idxA[posA] = grow[is_lo]
        normv = np.zeros((128, TA + TB), dtype=np.float32)
        normv[p_all[is_lo], colA[is_lo]] = en[is_lo]
        if TB > 0:
            idxB = np.zeros(TB * 128, dtype=np.int64)
            idxB[posB] = grow[~is_lo] - B_BASE
            normv[p_all[~is_lo], TA + colB[~is_lo]] = en[~is_lo]
            assert idxB.min() >= 0 and idxB.max() < cfg.WINDOW
        assert idxA.max() < cfg.WINDOW

        xk = np.zeros((cfg.F, NSHP), dtype=np.float32)
        xk[:, :NSH] = np.asarray(x[k * NSH:(k + 1) * NSH], np.float32)[orders[k]].T

        im = {
            "xT": np.ascontiguousarray(xk),
            "W1T": np.ascontiguousarray(np.asarray(W1, np.float32).T),
            "b1": np.asarray(b1, np.float32).reshape(1, cfg.H),
            "W2T": np.ascontiguousarray(np.asarray(W2, np.float32).T),
            "b2": np.asarray(b2, np.float32).reshape(1, cfg.C),
            "attl": np.asarray(att_l, np.float32).reshape(1, -1),
            "attr": np.asarray(att_r, np.float32).reshape(1, -1),
            "idxA": wrap16(idxA.astype(np.int16)),
            "normv": normv,
        }
        if TB > 0:
            im["idxB"] = wrap16(idxB.astype(np.int16))
        in_maps.append(im)
    return in_maps, orders, (CA.tolist(), CB.tolist())


def build_nc(cfg: Cfg, CACB):
    CA, CB = (np.asarray(v, dtype=np.int64) for v in CACB)
    TPC, H, C, L = cfg.TPC, cfg.H, cfg.C, cfg.L
    offA = np.zeros(TPC + 1, dtype=np.int64)
    np.cumsum(CA, out=offA[1:])
    offB = np.zeros(TPC + 1, dtype=np.int64)
    np.cumsum(CB, out=offB[1:])
    TA, TB = int(offA[-1]), int(offB[-1])
    NSLC = cfg.F // 128

    nc = bacc.Bacc("TRN2", target_bir_lowering=False, debug=False,
                   num_devices=cfg.M)
    xT_h = nc.dram_tensor("xT", [cfg.F, cfg.NSHP], F32, kind="ExternalInput")
    W1T_h = nc.dram_tensor("W1T", [cfg.F, H], F32, kind="ExternalInput")
    b1_h = nc.dram_tensor("b1", [1, H], F32, kind="ExternalInput")
    W2T_h = nc.dram_tensor("W2T", [H, C], F32, kind="ExternalInput")
    b2_h = nc.dram_tensor("b2", [1, C], F32, kind="ExternalInput")
    attl_h = nc.dram_tensor("attl", [1, L * H], F32, kind="ExternalInput")
    attr_h = nc.dram_tensor("attr", [1, L * H], F32, kind="ExternalInput")
    idxA_h = nc.dram_tensor("idxA", [128, 8 * TA], I16, kind="ExternalInput")
    if TB > 0:
        idxB_h = nc.dram_tensor("idxB", [128, 8 * TB], I16, kind="ExternalInput")
    normv_h = nc.dram_tensor("normv", [128, TA + TB], F32, kind="ExternalInput")
    out_h = nc.dram_tensor("out", [cfg.NSHP, C], F32, kind="ExternalOutput")

    # chunks: consecutive tiles with both window spans <= CHUNK_COLS
    chunks = []  # (t0, t1)
    t0 = 0
    for t in range(TPC + 1):
        if t == TPC or (t > t0 and
                        (offA[t] - offA[t0] + CA[t] > cfg.CHUNK_COLS or
                         offB[t] - offB[t0] + CB[t] > cfg.CHUNK_COLS)):
            if t > t0:
                chunks.append((t0, t))
            t0 = t

    with tile.TileContext(nc) as tc:
        with tc.tile_pool(name="dram", bufs=2, space="DRAM") as dram, \
             tc.tile_pool(name="pers", bufs=1) as pers, \
             tc.tile_pool(name="gpool", bufs=2) as gpool, \
             tc.tile_pool(name="cpool", bufs=3) as cpool, \
             tc.tile_pool(name="mpool", bufs=4) as mpool, \
             tc.tile_pool(name="spool", bufs=2) as spool, \
             tc.tile_pool(name="apsum", bufs=2, space="PSUM") as apsum, \
             tc.tile_pool(name="bpsum", bufs=2, space="PSUM") as bpsum:

            ones = pers.tile([1, 128], F32)
            nc.vector.memset(ones[:], 1.0)
            ident = pers.tile([128, 128], F32)
            make_identity(nc, ident[:])
            identb = pers.tile([128, 128], BF16)
            nc.vector.tensor_copy(identb[:], ident[:])
            b1s = pers.tile([1, H], F32)
            nc.sync.dma_start(b1s[:], b1_h[:])
            b2s = pers.tile([1, C], F32)
            nc.sync.dma_start(b2s[:], b2_h[:])
            W2Ts = pers.tile([H, C], F32)
            nc.sync.dma_start(W2Ts[:], W2T_h[:])
            W1Ts = pers.tile([128, NSLC, H], F32)
            nc.sync.dma_start(W1Ts[:], W1T_h[:].rearrange("(s p) h -> p s h", p=128))
            attls = pers.tile([1, L * H], F32)
            nc.sync.dma_start(attls[:], attl_h[:])
            attrs = pers.tile([1, L * H], F32)
            nc.sync.dma_start(attrs[:], attr_h[:])
            idxA = pers.tile([128, 8 * TA], I16)
            nc.sync.dma_start(idxA[:], idxA_h[:])
            if TB > 0:
                idxB = pers.tile([128, 8 * TB], I16)
                nc.sync.dma_start(idxB[:], idxB_h[:])
            normv = pers.tile([128, TA + TB], F32)
            nc.sync.dma_start(normv[:], normv_h[:])

            attbc = pers.tile([128, max(2 * L, 1), H], F32)
            for l in range(L):
                for j, srcrow in enumerate((attls, attrs)):
                    bc = bpsum.tile([128, H], F32, tag="bc")
                    nc.tensor.matmul(bc[:], lhsT=ones[:],
                                     rhs=srcrow[0:1, l * H:(l + 1) * H],
                                     start=True, stop=True)
                    nc.vector.tensor_copy(attbc[:, 2 * l + j, :], bc[:])

            stage = pers.tile([128, TPC, 128], F32)
            nc.vector.memset(stage[:, :, H + 1:], 0.0)
            raw = pers.tile([128, TPC, H], F32)
            ar_sb = pers.tile([128, TPC], F32)
            outs = pers.tile([128, TPC, C], F32)
            mx_all = pers.tile([128, TPC], F32)
            se_all = pers.tile([128, TPC], F32)
            lse_all = pers.tile([128, TPC], F32)

            # ---- phase 0: h0 = relu(x @ W1.T + b1); al0/ar0
            XG = 4
            with tc.tile_pool(name="xpool", bufs=2) as xpool:
                for g0 in range(0, TPC, XG):
                    g1 = min(g0 + XG, TPC)
                    cw = (g1 - g0) * 128
                    xt = xpool.tile([128, NSLC, cw], F32, tag="xt")
                    nc.sync.dma_start(
                        xt[:, :, :],
                        xT_h[:, g0 * 128:g1 * 128].rearrange("(s p) c -> p s c", p=128))
                    for t in range(g0, g1):
                        lc = (t - g0) * 128
                        acc = apsum.tile([128, H], F32, tag="acc")
                        nc.tensor.matmul(acc[:], lhsT=ones[:], rhs=b1s[:],
                                         start=True, stop=False)
                        for s in range(NSLC):
                            nc.tensor.matmul(acc[:], lhsT=xt[:, s, lc:lc + 128],
                                             rhs=W1Ts[:, s, :],
                                             start=False, stop=(s == NSLC - 1))
                        nc.scalar.activation(stage[:, t, 0:H], acc[:], AF.Relu)
                        nc.vector.tensor_copy(raw[:, t, :], stage[:, t, 0:H])
                        if L > 0:
                            scr = cpool.tile([128, H], F32, tag="scr")
                            nc.vector.scalar_tensor_tensor(
                                out=scr[:], in0=stage[:, t, 0:H], scalar=1.0,
                                in1=attbc[:, 0, :], op0=OP.mult, op1=OP.mult,
                                accum_out=stage[:, t, H:H + 1])
                            scr2 = cpool.tile([128, H], F32, tag="scr")
                            nc.vector.scalar_tensor_tensor(
                                out=scr2[:], in0=stage[:, t, 0:H], scalar=1.0,
                                in1=attbc[:, 1, :], op0=OP.mult, op1=OP.mult,
                                accum_out=ar_sb[:, t:t + 1])

            # ---- layers
            for l in range(L):
                tbl_in = dram.tile([cfg.NSHP, 128], F32, tag="tbl_in")
                tbl_full = dram.tile([cfg.RF, 128], F32, tag="tbl_full",
                                     addr_space="Shared")
                nc.sync.dma_start(tbl_in[:].rearrange("(t p) e -> p t e", p=128),
                                  stage[:])
                nc.gpsimd.collective_compute(
                    "AllGather", OP.bypass,
                    replica_groups=[list(range(cfg.M))],
                    ins=[tbl_in.opt()], outs=[tbl_full.opt()])

                for (ct0, ct1) in chunks:
                    cA0, cA1 = int(offA[ct0]), int(offA[ct1])
                    cB0, cB1 = int(offB[ct0]), int(offB[ct1])
                    gA = gpool.tile([128, cfg.CHUNK_COLS, 128], F32, tag="gA")
                    nc.gpsimd.dma_gather(
                        out_ap=gA[:, :cA1 - cA0, :],
                        in_ap=(tbl_full[:cfg.WINDOW, :] if cfg.two_windows
                               else tbl_full[:, :]),
                        idxs_ap=idxA[:, 8 * cA0:8 * cA1],
                        num_idxs=128 * (cA1 - cA0),
                        num_idxs_reg=128 * (cA1 - cA0),
                        elem_size=128, single_packet=False)
                    if TB > 0 and cB1 > cB0:
                        gB = gpool.tile([128, cfg.CHUNK_COLS, 128], F32, tag="gB")
                        nc.gpsimd.dma_gather(
                            out_ap=gB[:, :cB1 - cB0, :],
                            in_ap=tbl_full[cfg.RF - cfg.WINDOW:, :],
                            idxs_ap=idxB[:, 8 * cB0:8 * cB1],
                            num_idxs=128 * (cB1 - cB0),
                            num_idxs_reg=128 * (cB1 - cB0),
                            elem_size=128, single_packet=False)
                    for t in range(ct0, ct1):
                        nA, nB = int(CA[t]), int(CB[t])
                        lcA = int(offA[t]) - cA0
                        lcB = int(offB[t]) - cB0
                        parts = [(gA, lcA, nA, int(offA[t]))]
                        if nB > 0:
                            parts.append((gB, lcB, nB, TA + int(offB[t])))
                        coeffs = []
                        for (gg, lc, nn, noff) in parts:
                            cf = cpool.tile([128, cfg.CHUNK_COLS], F32, tag="coeff")
                            nc.scalar.activation(cf[:, :nn], gg[:, lc:lc + nn, H],
                                                 AF.Tanh, bias=ar_sb[:, t:t + 1])
                            nc.vector.tensor_tensor(
                                out=cf[:, :nn], in0=cf[:, :nn],
                                in1=normv[:, noff:noff + nn], op=OP.mult)
                            coeffs.append(cf)
                        acc = apsum.tile([128, H], F32, tag="acc")
                        nblk = nA + nB
                        bi = 0
                        for (gg, lc, nn, _), cf in zip(parts, coeffs):
                            for b in range(nn):
                                msg = mpool.tile([128, H], BF16, tag="msg")
                                nc.vector.tensor_scalar(
                                    out=msg[:], in0=gg[:, lc + b, 0:H],
                                    scalar1=cf[:, b:b + 1], scalar2=None,
                                    op0=OP.mult)
                                nc.tensor.matmul(acc[:], lhsT=identb[:], rhs=msg[:],
                                                 start=(bi == 0),
                                                 stop=(bi == nblk - 1))
                                bi += 1
                        nc.vector.scalar_tensor_tensor(
                            out=stage[:, t, 0:H], in0=raw[:, t, :], scalar=cfg.EPS,
                            in1=acc[:], op0=OP.mult, op1=OP.add)
                        if l < L - 1:
                            scr = cpool.tile([128, H], F32, tag="scr")
                            nc.vector.scalar_tensor_tensor(
                                out=scr[:], in0=stage[:, t, 0:H], scalar=1.0,
                                in1=attbc[:, 2 * (l + 1), :], op0=OP.mult,
                                op1=OP.mult, accum_out=stage[:, t, H:H + 1])
                            scr2 = cpool.tile([128, H], F32, tag="scr")
                            nc.vector.scalar_tensor_tensor(
                                out=scr2[:], in0=stage[:, t, 0:H], scalar=1.0,
                                in1=attbc[:, 2 * (l + 1) + 1, :], op0=OP.mult,
                                op1=OP.mult, accum_out=ar_sb[:, t:t + 1])

            # ---- logits + log_softmax
            for t in range(TPC):
                tr = bpsum.tile([H, 128], F32, tag="tr")
                nc.tensor.transpose(out=tr[:], in_=stage[:, t, 0:H],
                                    identity=ident[:])
                htT = spool.tile([H, 128], F32, tag="htT")
                nc.vector.tensor_copy(htT[:], tr[:])
                lg = bpsum.tile([128, C], F32, tag="lg")
                nc.tensor.matmul(lg[:], lhsT=ones[:], rhs=b2s[:],
                                 start=True, stop=False)
                nc.tensor.matmul(lg[:], lhsT=htT[:], rhs=W2Ts[:],
                                 start=False, stop=True)
                nc.vector.tensor_reduce(out=mx_all[:, t:t + 1], in_=lg[:],
                                        axis=mybir.AxisListType.X, op=OP.max,
                                        negate=True)
                scr40 = cpool.tile([128, C], F32, tag="scr40")
                nc.scalar.activation(scr40[:], lg[:], AF.Exp,
                                     bias=mx_all[:, t:t + 1],
                                     accum_out=se_all[:, t:t + 1])
                nc.vector.tensor_copy(outs[:, t, :], lg[:])
            nc.scalar.activation(lse_all[:], se_all[:], AF.Ln)
            for t in range(TPC):
                nc.vector.tensor_scalar(
                    out=outs[:, t, :], in0=outs[:, t, :],
                    scalar1=mx_all[:, t:t + 1], scalar2=lse_all[:, t:t + 1],
                    op0=OP.add, op1=OP.subtract)
            nc.sync.dma_start(out_h[:].rearrange("(t p) c -> p t c", p=128),
                              outs[:])
    nc.compile()
    return nc


def run(cfg: Cfg, inputs: dict, trace: bool = False):
    in_maps, orders, CACB = host_prep(cfg, **inputs)
    nc = build_nc(cfg, CACB)
    res = bass_utils.run_bass_kernel_spmd(
        nc, in_maps, core_ids=list(range(cfg.M)), trace=trace)
    out = np.empty((cfg.N, cfg.C), dtype=np.float32)
    for k in range(cfg.M):
        out[k * cfg.NSH + orders[k]] = np.asarray(res.results[k]["out"],
                                                  np.float32)[:cfg.NSH]
    return out, res


def kernel(x, edge_index, W1, b1, W2, b2, att_l, att_r):
    cfg = Cfg()
    out, _ = run(cfg, dict(x=np.asarray(x, np.float32),
                           edge_index=np.asarray(edge_index),
                           W1=W1, b1=b1, W2=W2, b2=b2,
                           att_l=att_l, att_r=att_r))
    return out



# revision 9
# speedup vs baseline: 1.6924x; 1.6924x over previous
"""FAGCN (4-layer FAConv + lin1/lin2 + log_softmax) on 8 Trainium2 cores.

v2 — gather-descriptor-bound baseline reworked:
- bf16 h-table rows of 256B (h(64)|al|junk) halve gather + AllGather bytes.
- dma_gather round-robin over 4 SWDGE queues: desc-gen runs on Q7 core pair
  (2q, 2q+1), so 4 queues ~4x the descriptor throughput (was the bottleneck:
  78% GpSimd busy on queue 0 only).
- Self-loop messages computed on-chip (per-node, batched DVE) instead of
  gathered; removes 6.25k slots/core and shrinks per-node slot counts.
- (d, n2-n0) node ordering lowers shared CA/CB slot padding 1.48x -> 1.36x.
- Per-slot DVE scaling replaced by per-chunk broadcast (stride-0 AP) multiply;
  coeff tanh stays per-tile on ACT (ar as per-partition bias).
- Per-slot identity matmuls replaced by 8-slot-wide matmuls into a [128,512]
  PSUM bank per tile (slot k -> col block k%8; blocks summed by a batched
  DVE tree fold per 4-tile group).
- Phase 0 (x @ W1) in bf16.
"""
import numpy as np
from dataclasses import dataclass

import concourse.bass as bass
import concourse.bacc as bacc
import concourse.tile as tile
import concourse.mybir as mybir
from concourse import bass_utils
from concourse.masks import make_identity

F32 = mybir.dt.float32
BF16 = mybir.dt.bfloat16
I16 = mybir.dt.int16
AF = mybir.ActivationFunctionType
OP = mybir.AluOpType


@dataclass
class Cfg:
    N: int = 50000
    E: int = 800000
    F: int = 512
    H: int = 64
    C: int = 40
    L: int = 4
    EPS: float = 0.2
    M: int = 8           # cores
    CHUNK_COLS: int = 32
    WINDOW: int = 32768  # dma_gather int16 index limit
    QUEUES: int = 4      # SWDGE queues for gather desc-gen
    GT: int = 4          # tiles per psum fold group

    @property
    def NSH(self):
        return self.N // self.M

    @property
    def TPC(self):
        return (self.NSH + 127) // 128

    @property
    def NSHP(self):
        return self.TPC * 128

    @property
    def RF(self):
        return self.NSHP * self.M


def host_prep(cfg: Cfg, x, edge_index, W1, b1, W2, b2, att_l, att_r):
    """Shard + order nodes, build window-split gather/norm arrays (no loops)."""
    import ml_dtypes
    N, M, NSH, NSHP, TPC = cfg.N, cfg.M, cfg.NSH, cfg.NSHP, cfg.TPC
    src = np.asarray(edge_index[0], dtype=np.int64)
    dst = np.asarray(edge_index[1], dtype=np.int64)
    deg = np.bincount(dst, minlength=N).astype(np.float32) + 1.0  # + self loop
    dinv = (1.0 / np.sqrt(deg)).astype(np.float32)
    norm_e = (dinv[src] * dinv[dst]).astype(np.float32)
    core_of = dst // NSH
    B_BASE = cfg.RF - cfg.WINDOW

    per_core = []  # (es, dl, n0_, n1_)? computed per iteration
    for k in range(M):
        m = core_of == k
        per_core.append((src[m], dst[m] - k * NSH, norm_e[m]))

    def feats(grow_map):
        out = []
        for k in range(M):
            es, ds, _ = per_core[k]
            grow = grow_map[es]
            cls = np.where(grow >= cfg.WINDOW, 2,
                           np.where(grow >= B_BASE, 1, 0))
            n0 = np.bincount(ds[cls == 0], minlength=NSH)
            n2 = np.bincount(ds[cls == 2], minlength=NSH)
            d = np.bincount(ds, minlength=NSH)
            out.append((n0, n2, d))
        return out

    def grow_from(orders):
        gm = np.empty(N, dtype=np.int64)
        for k in range(M):
            inv = np.empty(NSH, dtype=np.int64)
            inv[orders[k]] = np.arange(NSH)
            gm[k * NSH:(k + 1) * NSH] = k * NSHP + inv
        return gm

    orders = [np.arange(NSH) for _ in range(M)]
    for _ in range(2):
        f = feats(grow_from(orders))
        orders = [np.lexsort((f[k][1] - f[k][0], -f[k][2])) for k in range(M)]
    grow_map = grow_from(orders)
    f = feats(grow_map)

    # shared per-tile window budgets
    A0 = np.zeros(TPC, dtype=np.int64)
    B2 = np.zeros(TPC, dtype=np.int64)
    D = np.zeros(TPC, dtype=np.int64)
    inv_orders = []
    for k in range(M):
        inv = np.empty(NSH, dtype=np.int64)
        inv[orders[k]] = np.arange(NSH)
        inv_orders.append(inv)
        n0o = np.zeros(NSHP, dtype=np.int64)
        n2o = np.zeros(NSHP, dtype=np.int64)
        do = np.zeros(NSHP, dtype=np.int64)
        n0o[:NSH] = f[k][0][orders[k]]
        n2o[:NSH] = f[k][1][orders[k]]
        do[:NSH] = f[k][2][orders[k]]
        A0 = np.maximum(A0, n0o.reshape(TPC, 128).max(1))
        B2 = np.maximum(B2, n2o.reshape(TPC, 128).max(1))
        D = np.maximum(D, do.reshape(TPC, 128).max(1))
    cost = np.maximum(A0 + B2, np.maximum(D, 4))
    CA = np.maximum(np.maximum(A0, cost - B2), 4)  # >=1 full 4-slot A group
    CB = np.maximum(cost - CA, B2)
    assert CA.max() <= cfg.CHUNK_COLS and CB.max() <= cfg.CHUNK_COLS, \
        (CA.max(), CB.max())
    offA = np.zeros(TPC + 1, dtype=np.int64)
    np.cumsum(CA, out=offA[1:])
    offB = np.zeros(TPC + 1, dtype=np.int64)
    np.cumsum(CB, out=offB[1:])
    TA, TB = int(offA[-1]), int(offB[-1])

    def wrap16(lst16):
        a = lst16.reshape(-1, 16).T.copy()
        return np.tile(a, (8, 1)).astype(np.int16)

    perm_f = None  # no feature permutation
    in_maps = []
    for k in range(M):
        es, ds, en = per_core[k]
        rk = inv_orders[k][ds]
        grow = grow_map[es]
        cls = np.where(grow >= cfg.WINDOW, 2,
                       np.where(grow >= B_BASE, 1, 0)).astype(np.int8)
        n0 = np.bincount(rk[cls == 0], minlength=NSHP)
        n1 = np.bincount(rk[cls == 1], minlength=NSHP)
        d = np.bincount(rk, minlength=NSHP)
        t_all0 = np.arange(NSHP) // 128
        nlo = np.minimum(CA[t_all0][np.arange(NSHP)], n0 + n1)
        nlo = np.maximum(nlo, n0)
        assert (d - nlo <= CB[t_all0]).all()

        o = np.lexsort((cls, rk))
        rk_s, en_s, grow_s, cls_s = rk[o], en[o], grow[o], cls[o]
        dl = np.bincount(rk_s, minlength=NSHP)
        run0 = np.repeat(np.cumsum(np.concatenate([[0], dl]))[:-1], dl)
        j = np.arange(len(rk_s)) - run0
        is_lo = j < nlo[rk_s]
        p_all = rk_s % 128
        t_all = rk_s // 128
        colA = offA[t_all] + j
        colB = offB[t_all] + (j - nlo[rk_s])
        posA = colA[is_lo] * 128 + p_all[is_lo]
        posB = colB[~is_lo] * 128 + p_all[~is_lo]

        idxA = np.zeros(TA * 128, dtype=np.int64)
        idxA[posA] = grow_s[is_lo]
        normv = np.zeros((128, TA + TB), dtype=np.float32)
        normv[p_all[is_lo], colA[is_lo]] = en_s[is_lo]
        assert idxA.max() < cfg.WINDOW
        if TB > 0:
            idxB = np.zeros(TB * 128, dtype=np.int64)
            idxB[posB] = grow_s[~is_lo] - B_BASE
            normv[p_all[~is_lo], TA + colB[~is_lo]] = en_s[~is_lo]
            assert idxB.min() >= 0 and idxB.max() < cfg.WINDOW

        xk = np.zeros((cfg.F, NSHP), dtype=ml_dtypes.bfloat16)
        xk[:, :NSH] = np.asarray(x[k * NSH:(k + 1) * NSH],
                                 np.float32)[orders[k]].T.astype(
                                     ml_dtypes.bfloat16)
        dinv2 = np.zeros((128, TPC), dtype=np.float32)
        dk = dinv[k * NSH:(k + 1) * NSH][orders[k]] ** 2
        dinv2[:, :] = np.pad(dk, (0, NSHP - NSH)).reshape(TPC, 128).T

        im = {
            "xT": np.ascontiguousarray(xk),
            "W1T": np.ascontiguousarray(
                np.asarray(W1, np.float32).T.astype(ml_dtypes.bfloat16)),
            "b1": np.asarray(b1, np.float32).reshape(1, cfg.H).astype(
                ml_dtypes.bfloat16),
            "W2T": np.ascontiguousarray(np.asarray(W2, np.float32).T),
            "b2": np.asarray(b2, np.float32).reshape(1, cfg.C),
            "attl": np.asarray(att_l, np.float32).reshape(1, -1),
            "attr": np.asarray(att_r, np.float32).reshape(1, -1),
            "idxA": wrap16(idxA.astype(np.int16)),
            "normv": normv.astype(ml_dtypes.bfloat16),
            "dinv2": dinv2,
        }
        if TB > 0:
            im["idxB"] = wrap16(idxB.astype(np.int16))
        in_maps.append(im)
    return in_maps, orders, (CA.tolist(), CB.tolist())


def build_nc(cfg: Cfg, CACB):
    CA, CB = (np.asarray(v, dtype=np.int64) for v in CACB)
    TPC, H, C, L = cfg.TPC, cfg.H, cfg.C, cfg.L
    offA = np.zeros(TPC + 1, dtype=np.int64)
    np.cumsum(CA, out=offA[1:])
    offB = np.zeros(TPC + 1, dtype=np.int64)
    np.cumsum(CB, out=offB[1:])
    TA, TB = int(offA[-1]), int(offB[-1])
    NSLC = cfg.F // 128

    nc = bacc.Bacc("TRN2", target_bir_lowering=False, debug=False,
                   num_devices=cfg.M, num_swdge_queues=cfg.QUEUES)
    xT_h = nc.dram_tensor("xT", [cfg.F, cfg.NSHP], BF16, kind="ExternalInput")
    W1T_h = nc.dram_tensor("W1T", [cfg.F, H], BF16, kind="ExternalInput")
    b1_h = nc.dram_tensor("b1", [1, H], BF16, kind="ExternalInput")
    W2T_h = nc.dram_tensor("W2T", [H, C], F32, kind="ExternalInput")
    b2_h = nc.dram_tensor("b2", [1, C], F32, kind="ExternalInput")
    attl_h = nc.dram_tensor("attl", [1, L * H], F32, kind="ExternalInput")
    attr_h = nc.dram_tensor("attr", [1, L * H], F32, kind="ExternalInput")
    idxA_h = nc.dram_tensor("idxA", [128, 8 * TA], I16, kind="ExternalInput")
    if TB > 0:
        idxB_h = nc.dram_tensor("idxB", [128, 8 * TB], I16, kind="ExternalInput")
    normv_h = nc.dram_tensor("normv", [128, TA + TB], BF16, kind="ExternalInput")
    dinv2_h = nc.dram_tensor("dinv2", [128, TPC], F32, kind="ExternalInput")
    out_h = nc.dram_tensor("out", [cfg.NSHP, C], F32, kind="ExternalOutput")

    # chunks: consecutive tiles with both window spans <= CHUNK_COLS
    chunks = []
    t0 = 0
    for t in range(TPC + 1):
        if t == TPC or (t > t0 and
                        (offA[t] - offA[t0] + CA[t] > cfg.CHUNK_COLS or
                         offB[t] - offB[t0] + CB[t] > cfg.CHUNK_COLS)):
            if t > t0:
                chunks.append((t0, t))
            t0 = t
    chunk_of = {}
    for ci, (a, b) in enumerate(chunks):
        for t in range(a, b):
            chunk_of[t] = ci

    with tile.TileContext(nc) as tc:
        with tc.tile_pool(name="dram", bufs=2, space="DRAM") as dram, \
             tc.tile_pool(name="pers", bufs=1) as pers, \
             tc.tile_pool(name="gpool", bufs=4) as gpool, \
             tc.tile_pool(name="cpool", bufs=4) as cpool, \
             tc.tile_pool(name="mpool", bufs=6) as mpool, \
             tc.tile_pool(name="fpool", bufs=2) as fpool:

            onesb = pers.tile([1, 128], BF16)
            nc.vector.memset(onesb[:], 1.0)
            ones = pers.tile([1, 128], F32)
            nc.vector.memset(ones[:], 1.0)
            ident = pers.tile([128, 128], F32)
            make_identity(nc, ident[:])
            identb = pers.tile([128, 128], BF16)
            nc.vector.tensor_copy(identb[:], ident[:])
            b1s = pers.tile([1, H], BF16)
            nc.sync.dma_start(b1s[:], b1_h[:])
            b2s = pers.tile([1, C], F32)
            nc.sync.dma_start(b2s[:], b2_h[:])
            W2Ts = pers.tile([H, C], F32)
            nc.sync.dma_start(W2Ts[:], W2T_h[:])
            W1Ts = pers.tile([128, NSLC, H], BF16)
            nc.sync.dma_start(W1Ts[:], W1T_h[:].rearrange("(s p) h -> p s h", p=128))
            attls = pers.tile([1, L * H], F32)
            nc.sync.dma_start(attls[:], attl_h[:])
            attrs = pers.tile([1, L * H], F32)
            nc.sync.dma_start(attrs[:], attr_h[:])
            idxA = pers.tile([128, 8 * TA], I16)
            nc.sync.dma_start(idxA[:], idxA_h[:])
            if TB > 0:
                idxB = pers.tile([128, 8 * TB], I16)
                nc.sync.dma_start(idxB[:], idxB_h[:])
            normv = pers.tile([128, TA + TB], BF16)
            nc.sync.dma_start(normv[:], normv_h[:])
            dinv2 = pers.tile([128, TPC], F32)
            nc.sync.dma_start(dinv2[:], dinv2_h[:])

            attbc = pers.tile([128, max(2 * L, 1), H], F32)
            with tc.tile_pool(name="bpsum", bufs=2, space="PSUM") as bpsum:
                for l in range(L):
                    for j, srcrow in enumerate((attls, attrs)):
                        bc = bpsum.tile([128, H], F32, tag="bc")
                        nc.tensor.matmul(bc[:], lhsT=ones[:],
                                         rhs=srcrow[0:1, l * H:(l + 1) * H],
                                         start=True, stop=True)
                        nc.vector.tensor_copy(attbc[:, 2 * l + j, :], bc[:])

            stage = pers.tile([128, TPC, H], F32)
            raw = pers.tile([128, TPC, H], F32)
            al_sb = pers.tile([128, TPC], F32)
            ar_sb = pers.tile([128, TPC], F32)
            export = pers.tile([128, TPC, H + 1], BF16)
            outs = pers.tile([128, TPC, C], F32)
            mx_all = pers.tile([128, TPC], F32)
            se_all = pers.tile([128, TPC], F32)
            lse_all = pers.tile([128, TPC], F32)

            def node_scores(l):
                """al/ar for layer l per tile (accum STT) + export al col."""
                for t in range(TPC):
                    scr = cpool.tile([128, H], F32, tag="scr")
                    nc.vector.scalar_tensor_tensor(
                        out=scr[:], in0=stage[:, t, :], scalar=1.0,
                        in1=attbc[:, 2 * l, :], op0=OP.mult, op1=OP.mult,
                        accum_out=al_sb[:, t:t + 1])
                    scr2 = cpool.tile([128, H], F32, tag="scr")
                    nc.vector.scalar_tensor_tensor(
                        out=scr2[:], in0=stage[:, t, :], scalar=1.0,
                        in1=attbc[:, 2 * l + 1, :], op0=OP.mult, op1=OP.mult,
                        accum_out=ar_sb[:, t:t + 1])

            def do_export():
                nc.vector.tensor_copy(export[:, :, 0:H], stage[:])
                nc.vector.tensor_copy(export[:, :, H:H + 1],
                                      al_sb[:].unsqueeze(2))

            # ---- phase 0: h0 = relu(x @ W1.T + b1)
            XG = 4
            with tc.tile_pool(name="xpool", bufs=2) as xpool, \
                 tc.tile_pool(name="ppsum", bufs=4, space="PSUM") as ppsum:
                for g0 in range(0, TPC, XG):
                    g1 = min(g0 + XG, TPC)
                    cw = (g1 - g0) * 128
                    xt = xpool.tile([128, NSLC, cw], BF16, tag="xt")
                    nc.sync.dma_start(
                        xt[:, :, :],
                        xT_h[:, g0 * 128:g1 * 128].rearrange("(s p) c -> p s c", p=128))
                    for t in range(g0, g1):
                        lc = (t - g0) * 128
                        acc = ppsum.tile([128, H], F32, tag="acc")
                        nc.tensor.matmul(acc[:], lhsT=onesb[:], rhs=b1s[:],
                                         start=True, stop=False)
                        for s in range(NSLC):
                            nc.tensor.matmul(acc[:], lhsT=xt[:, s, lc:lc + 128],
                                             rhs=W1Ts[:, s, :],
                                             start=False, stop=(s == NSLC - 1))
                        nc.scalar.activation(stage[:, t, :], acc[:], AF.Relu)
                        nc.vector.tensor_copy(raw[:, t, :], stage[:, t, :])
            node_scores(0)
            do_export()

            # ---- layers
            qctr = 0
            layer_psum = tc.tile_pool(name="qpsum", bufs=2, space="PSUM")
            qpsum = layer_psum.__enter__()
            for l in range(L):
                tbl_in = dram.tile([cfg.NSHP, 128], BF16, tag="tbl_in")
                tbl_full = dram.tile([cfg.RF, 128], BF16, tag="tbl_full",
                                     addr_space="Shared")
                nc.sync.dma_start(
                    tbl_in[:].rearrange("(t p) e -> p t e", p=128)[:, :, 0:H + 1],
                    export[:])
                nc.gpsimd.collective_compute(
                    "AllGather", OP.bypass,
                    replica_groups=[list(range(cfg.M))],
                    ins=[tbl_in.opt()], outs=[tbl_full.opt()])

                # self-loop coefficient per node
                cs = cpool.tile([128, TPC], F32, tag="cs")
                nc.vector.tensor_tensor(out=cs[:], in0=al_sb[:], in1=ar_sb[:],
                                        op=OP.add)
                nc.scalar.activation(cs[:], cs[:], AF.Tanh)
                nc.vector.tensor_tensor(out=cs[:], in0=cs[:], in1=dinv2[:],
                                        op=OP.mult)

                cur_chunk = -1
                gA = gB = msgA = msgB = None
                po = None
                g_t0 = 0

                def open_chunk(ci):
                    nonlocal gA, gB, msgA, msgB, qctr
                    ct0, ct1 = chunks[ci]
                    cA0, cA1 = int(offA[ct0]), int(offA[ct1])
                    cB0, cB1 = int(offB[ct0]), int(offB[ct1])
                    spanA, spanB = cA1 - cA0, cB1 - cB0
                    gA = gpool.tile([128, cfg.CHUNK_COLS, 128], BF16, tag="gA")
                    nc.gpsimd.dma_gather(
                        out_ap=gA[:, :spanA, :],
                        in_ap=tbl_full[:cfg.WINDOW, :],
                        idxs_ap=idxA[:, 8 * cA0:8 * cA1],
                        num_idxs=128 * spanA, num_idxs_reg=128 * spanA,
                        elem_size=128, single_packet=False,
                        queue_num=qctr % cfg.QUEUES)
                    qctr += 1
                    if TB > 0 and spanB > 0:
                        gB = gpool.tile([128, cfg.CHUNK_COLS, 128], BF16,
                                        tag="gB")
                        nc.gpsimd.dma_gather(
                            out_ap=gB[:, :spanB, :],
                            in_ap=tbl_full[cfg.RF - cfg.WINDOW:, :],
                            idxs_ap=idxB[:, 8 * cB0:8 * cB1],
                            num_idxs=128 * spanB, num_idxs_reg=128 * spanB,
                            elem_size=128, single_packet=False,
                            queue_num=qctr % cfg.QUEUES)
                        qctr += 1
                    # coeff: tanh(al_src + ar_dst) per tile, then * norm batched
                    cfA = cpool.tile([128, cfg.CHUNK_COLS], BF16, tag="cfA")
                    cfB = cpool.tile([128, cfg.CHUNK_COLS], BF16, tag="cfB")
                    for t in range(ct0, ct1):
                        nA = int(CA[t])
                        lcA = int(offA[t]) - cA0
                        nc.scalar.activation(cfA[:, lcA:lcA + nA],
                                             gA[:, lcA:lcA + nA, H],
                                             AF.Tanh, bias=ar_sb[:, t:t + 1])
                        nB = int(CB[t])
                        if nB > 0:
                            lcB = int(offB[t]) - cB0
                            nc.scalar.activation(cfB[:, lcB:lcB + nB],
                                                 gB[:, lcB:lcB + nB, H],
                                                 AF.Tanh, bias=ar_sb[:, t:t + 1])
                    nc.vector.tensor_tensor(
                        out=cfA[:, :spanA], in0=cfA[:, :spanA],
                        in1=normv[:, cA0:cA1], op=OP.mult)
                    msgA = mpool.tile([128, cfg.CHUNK_COLS, H], BF16, tag="mA")
                    nc.vector.tensor_tensor(
                        out=msgA[:, :spanA, :], in0=gA[:, :spanA, 0:H],
                        in1=cfA[:, :spanA].unsqueeze(2).broadcast_to(
                            (128, spanA, H)), op=OP.mult)
                    if spanB > 0:
                        nc.vector.tensor_tensor(
                            out=cfB[:, :spanB], in0=cfB[:, :spanB],
                            in1=normv[:, TA + cB0:TA + cB1], op=OP.mult)
                        msgB = mpool.tile([128, cfg.CHUNK_COLS, H], BF16,
                                          tag="mB")
                        nc.vector.tensor_tensor(
                            out=msgB[:, :spanB, :], in0=gB[:, :spanB, 0:H],
                            in1=cfB[:, :spanB].unsqueeze(2).broadcast_to(
                                (128, spanB, H)), op=OP.mult)

                def fold_group(t0g, t1g):
                    n = t1g - t0g
                    # one PSUM operand per DVE op: copy, then accumulate
                    c0 = fpool.tile([128, cfg.GT, 128], F32, tag="c0")
                    nc.vector.tensor_copy(c0[:, :n, :],
                                          po[:, t0g - g_t0:t1g - g_t0, 0:128])
                    nc.vector.tensor_tensor(out=c0[:, :n, :],
                                            in0=c0[:, :n, :],
                                            in1=po[:, t0g - g_t0:t1g - g_t0, 128:256],
                                            op=OP.add)
                    f3 = fpool.tile([128, cfg.GT, H], F32, tag="f3")
                    nc.vector.tensor_tensor(out=f3[:, :n, :],
                                            in0=c0[:, :n, 0:H],
                                            in1=c0[:, :n, H:128], op=OP.add)
                    ms = fpool.tile([128, cfg.GT, H], F32, tag="ms")
                    nc.vector.tensor_tensor(
                        out=ms[:, :n, :], in0=stage[:, t0g:t1g, :],
                        in1=cs[:, t0g:t1g].unsqueeze(2).broadcast_to(
                            (128, n, H)), op=OP.mult)
                    nc.vector.scalar_tensor_tensor(
                        out=stage[:, t0g:t1g, :], in0=raw[:, t0g:t1g, :],
                        scalar=cfg.EPS, in1=f3[:, :n, :],
                        op0=OP.mult, op1=OP.add)
                    nc.vector.tensor_tensor(out=stage[:, t0g:t1g, :],
                                            in0=stage[:, t0g:t1g, :],
                                            in1=ms[:, :n, :], op=OP.add)

                for t in range(TPC):
                    ci = chunk_of[t]
                    if ci != cur_chunk:
                        open_chunk(ci)
                        cur_chunk = ci
                    if t % cfg.GT == 0:
                        g_t0 = t
                        po = qpsum.tile([128, cfg.GT, 256], F32, tag="po")
                    j = t - g_t0
                    ct0 = chunks[ci][0]
                    lcA = int(offA[t]) - int(offA[ct0])
                    lcB = int(offB[t]) - int(offB[ct0])
                    nA, nB = int(CA[t]), int(CB[t])
                    W = 4  # slots per wide matmul (256 psum cols)
                    ops = []  # (msg, lc, nslots) full groups first, then ragged
                    for g in range(nA // W):
                        ops.append((msgA, lcA + W * g, W))
                    for g in range(nB // W):
                        ops.append((msgB, lcB + W * g, W))
                    if nA % W:
                        ops.append((msgA, lcA + W * (nA // W), nA % W))
                    if nB % W:
                        ops.append((msgB, lcB + W * (nB // W), nB % W))
                    assert ops and ops[0][2] == W
                    for oi, (mm, lc, ns) in enumerate(ops):
                        cols = 64 * ns
                        nc.tensor.matmul(
                            po[:, j, 256 - cols:256],
                            lhsT=identb[:],
                            rhs=mm[:, lc:lc + ns, :],
                            start=(oi == 0), stop=(oi == len(ops) - 1),
                            skip_group_check=True)
                    if t % cfg.GT == cfg.GT - 1 or t == TPC - 1:
                        fold_group(g_t0, t + 1)

                if l < L - 1:
                    node_scores(l + 1)
                    do_export()
            layer_psum.__exit__(None, None, None)

            # ---- logits + log_softmax
            with tc.tile_pool(name="spool", bufs=2) as spool, \
                 tc.tile_pool(name="lpsum", bufs=4, space="PSUM") as lpsum:
                for t in range(TPC):
                    tr = lpsum.tile([H, 128], F32, tag="tr")
                    nc.tensor.transpose(out=tr[:], in_=stage[:, t, :],
                                        identity=ident[:])
                    htT = spool.tile([H, 128], F32, tag="htT")
                    nc.vector.tensor_copy(htT[:], tr[:])
                    lg = lpsum.tile([128, C], F32, tag="lg")
                    nc.tensor.matmul(lg[:], lhsT=ones[:], rhs=b2s[:],
                                     start=True, stop=False)
                    nc.tensor.matmul(lg[:], lhsT=htT[:], rhs=W2Ts[:],
                                     start=False, stop=True)
                    nc.vector.tensor_reduce(out=mx_all[:, t:t + 1], in_=lg[:],
                                            axis=mybir.AxisListType.X, op=OP.max,
                                            negate=True)
                    scr40 = cpool.tile([128, C], F32, tag="scr40")
                    nc.scalar.activation(scr40[:], lg[:], AF.Exp,
                                         bias=mx_all[:, t:t + 1],
                                         accum_out=se_all[:, t:t + 1])
                    nc.vector.tensor_copy(outs[:, t, :], lg[:])
                nc.scalar.activation(lse_all[:], se_all[:], AF.Ln)
                for t in range(TPC):
                    nc.vector.tensor_scalar(
                        out=outs[:, t, :], in0=outs[:, t, :],
                        scalar1=mx_all[:, t:t + 1], scalar2=lse_all[:, t:t + 1],
                        op0=OP.add, op1=OP.subtract)
                nc.sync.dma_start(out_h[:].rearrange("(t p) c -> p t c", p=128),
                                  outs[:])
    nc.compile()
    return nc


def run(cfg: Cfg, inputs: dict, trace: bool = False):
    in_maps, orders, CACB = host_prep(cfg, **inputs)
    nc = build_nc(cfg, CACB)
    res = bass_utils.run_bass_kernel_spmd(
        nc, in_maps, core_ids=list(range(cfg.M)), trace=trace)
    out = np.empty((cfg.N, cfg.C), dtype=np.float32)
    for k in range(cfg.M):
        out[k * cfg.NSH + orders[k]] = np.asarray(res.results[k]["out"],
                                                  np.float32)[:cfg.NSH]
    return out, res


def kernel(x, edge_index, W1, b1, W2, b2, att_l, att_r):
    cfg = Cfg()
    out, _ = run(cfg, dict(x=np.asarray(x, np.float32),
                           edge_index=np.asarray(edge_index),
                           W1=W1, b1=b1, W2=W2, b2=b2,
                           att_l=att_l, att_r=att_r))
    return out


# revision 13
# speedup vs baseline: 1.7068x; 1.0085x over previous
"""FAGCN (4-layer FAConv + lin1/lin2 + log_softmax) on 8 Trainium2 cores.

v2 — gather-descriptor-bound baseline reworked:
- bf16 h-table rows of 256B (h(64)|al|junk) halve gather + AllGather bytes.
- dma_gather round-robin over 4 SWDGE queues: desc-gen runs on Q7 core pair
  (2q, 2q+1), so 4 queues ~4x the descriptor throughput (was the bottleneck:
  78% GpSimd busy on queue 0 only).
- Self-loop messages computed on-chip (per-node, batched DVE) instead of
  gathered; removes 6.25k slots/core and shrinks per-node slot counts.
- (d, n2-n0) node ordering lowers shared CA/CB slot padding 1.48x -> 1.36x.
- Per-slot DVE scaling replaced by per-chunk broadcast (stride-0 AP) multiply;
  coeff tanh stays per-tile on ACT (ar as per-partition bias).
- Per-slot identity matmuls replaced by 8-slot-wide matmuls into a [128,512]
  PSUM bank per tile (slot k -> col block k%8; blocks summed by a batched
  DVE tree fold per 4-tile group).
- Phase 0 (x @ W1) in bf16.
"""
import numpy as np
from dataclasses import dataclass

import concourse.bass as bass
import concourse.bacc as bacc
import concourse.tile as tile
import concourse.mybir as mybir
from concourse import bass_utils
from concourse.masks import make_identity

F32 = mybir.dt.float32
BF16 = mybir.dt.bfloat16
I16 = mybir.dt.int16
AF = mybir.ActivationFunctionType
OP = mybir.AluOpType


@dataclass
class Cfg:
    N: int = 50000
    E: int = 800000
    F: int = 512
    H: int = 64
    C: int = 40
    L: int = 4
    EPS: float = 0.2
    M: int = 8           # cores
    CHUNK_COLS: int = 32
    WINDOW: int = 32768  # dma_gather int16 index limit
    QUEUES: int = 4      # SWDGE queues for gather desc-gen
    GT: int = 4          # tiles per psum fold group

    @property
    def NSH(self):
        return self.N // self.M

    @property
    def TPC(self):
        return (self.NSH + 127) // 128

    @property
    def NSHP(self):
        return self.TPC * 128

    @property
    def RF(self):
        return self.NSHP * self.M


def host_prep(cfg: Cfg, x, edge_index, W1, b1, W2, b2, att_l, att_r):
    """Shard + order nodes, build window-split gather/norm arrays (no loops)."""
    import ml_dtypes
    N, M, NSH, NSHP, TPC = cfg.N, cfg.M, cfg.NSH, cfg.NSHP, cfg.TPC
    src = np.asarray(edge_index[0], dtype=np.int64)
    dst = np.asarray(edge_index[1], dtype=np.int64)
    deg = np.bincount(dst, minlength=N).astype(np.float32) + 1.0  # + self loop
    dinv = (1.0 / np.sqrt(deg)).astype(np.float32)
    norm_e = (dinv[src] * dinv[dst]).astype(np.float32)
    core_of = dst // NSH
    B_BASE = cfg.RF - cfg.WINDOW

    per_core = []  # (es, dl, n0_, n1_)? computed per iteration
    for k in range(M):
        m = core_of == k
        per_core.append((src[m], dst[m] - k * NSH, norm_e[m]))

    def feats(grow_map):
        out = []
        for k in range(M):
            es, ds, _ = per_core[k]
            grow = grow_map[es]
            cls = np.where(grow >= cfg.WINDOW, 2,
                           np.where(grow >= B_BASE, 1, 0))
            n0 = np.bincount(ds[cls == 0], minlength=NSH)
            n2 = np.bincount(ds[cls == 2], minlength=NSH)
            d = np.bincount(ds, minlength=NSH)
            out.append((n0, n2, d))
        return out

    def grow_from(orders):
        gm = np.empty(N, dtype=np.int64)
        for k in range(M):
            inv = np.empty(NSH, dtype=np.int64)
            inv[orders[k]] = np.arange(NSH)
            gm[k * NSH:(k + 1) * NSH] = k * NSHP + inv
        return gm

    orders = [np.arange(NSH) for _ in range(M)]
    for _ in range(2):
        f = feats(grow_from(orders))
        orders = [np.lexsort((f[k][1] - f[k][0], -f[k][2])) for k in range(M)]
    grow_map = grow_from(orders)
    f = feats(grow_map)

    # shared per-tile window budgets
    A0 = np.zeros(TPC, dtype=np.int64)
    B2 = np.zeros(TPC, dtype=np.int64)
    D = np.zeros(TPC, dtype=np.int64)
    inv_orders = []
    for k in range(M):
        inv = np.empty(NSH, dtype=np.int64)
        inv[orders[k]] = np.arange(NSH)
        inv_orders.append(inv)
        n0o = np.zeros(NSHP, dtype=np.int64)
        n2o = np.zeros(NSHP, dtype=np.int64)
        do = np.zeros(NSHP, dtype=np.int64)
        n0o[:NSH] = f[k][0][orders[k]]
        n2o[:NSH] = f[k][1][orders[k]]
        do[:NSH] = f[k][2][orders[k]]
        A0 = np.maximum(A0, n0o.reshape(TPC, 128).max(1))
        B2 = np.maximum(B2, n2o.reshape(TPC, 128).max(1))
        D = np.maximum(D, do.reshape(TPC, 128).max(1))
    cost = np.maximum(A0 + B2, np.maximum(D, 4))
    CA = np.maximum(np.maximum(A0, cost - B2), 4)  # >=1 full 4-slot A group
    CB = np.maximum(cost - CA, B2)
    assert CA.max() <= cfg.CHUNK_COLS and CB.max() <= cfg.CHUNK_COLS, \
        (CA.max(), CB.max())
    offA = np.zeros(TPC + 1, dtype=np.int64)
    np.cumsum(CA, out=offA[1:])
    offB = np.zeros(TPC + 1, dtype=np.int64)
    np.cumsum(CB, out=offB[1:])
    TA, TB = int(offA[-1]), int(offB[-1])

    def wrap16(lst16):
        a = lst16.reshape(-1, 16).T.copy()
        return np.tile(a, (8, 1)).astype(np.int16)

    perm_f = None  # no feature permutation
    in_maps = []
    for k in range(M):
        es, ds, en = per_core[k]
        rk = inv_orders[k][ds]
        grow = grow_map[es]
        cls = np.where(grow >= cfg.WINDOW, 2,
                       np.where(grow >= B_BASE, 1, 0)).astype(np.int8)
        n0 = np.bincount(rk[cls == 0], minlength=NSHP)
        n1 = np.bincount(rk[cls == 1], minlength=NSHP)
        d = np.bincount(rk, minlength=NSHP)
        t_all0 = np.arange(NSHP) // 128
        nlo = np.minimum(CA[t_all0][np.arange(NSHP)], n0 + n1)
        nlo = np.maximum(nlo, n0)
        assert (d - nlo <= CB[t_all0]).all()

        o = np.lexsort((cls, rk))
        rk_s, en_s, grow_s, cls_s = rk[o], en[o], grow[o], cls[o]
        dl = np.bincount(rk_s, minlength=NSHP)
        run0 = np.repeat(np.cumsum(np.concatenate([[0], dl]))[:-1], dl)
        j = np.arange(len(rk_s)) - run0
        is_lo = j < nlo[rk_s]
        p_all = rk_s % 128
        t_all = rk_s // 128
        colA = offA[t_all] + j
        colB = offB[t_all] + (j - nlo[rk_s])
        posA = colA[is_lo] * 128 + p_all[is_lo]
        posB = colB[~is_lo] * 128 + p_all[~is_lo]

        idxA = np.zeros(TA * 128, dtype=np.int64)
        idxA[posA] = grow_s[is_lo]
        normv = np.zeros((128, TA + TB), dtype=np.float32)
        normv[p_all[is_lo], colA[is_lo]] = en_s[is_lo]
        assert idxA.max() < cfg.WINDOW
        if TB > 0:
            idxB = np.zeros(TB * 128, dtype=np.int64)
            idxB[posB] = grow_s[~is_lo] - B_BASE
            normv[p_all[~is_lo], TA + colB[~is_lo]] = en_s[~is_lo]
            assert idxB.min() >= 0 and idxB.max() < cfg.WINDOW

        xk = np.zeros((cfg.F, NSHP), dtype=ml_dtypes.bfloat16)
        xk[:, :NSH] = np.asarray(x[k * NSH:(k + 1) * NSH],
                                 np.float32)[orders[k]].T.astype(
                                     ml_dtypes.bfloat16)
        dinv2 = np.zeros((128, TPC), dtype=np.float32)
        dk = dinv[k * NSH:(k + 1) * NSH][orders[k]] ** 2
        dinv2[:, :] = np.pad(dk, (0, NSHP - NSH)).reshape(TPC, 128).T

        im = {
            "xT": np.ascontiguousarray(xk),
            "W1T": np.ascontiguousarray(
                np.asarray(W1, np.float32).T.astype(ml_dtypes.bfloat16)),
            "b1": np.asarray(b1, np.float32).reshape(1, cfg.H).astype(
                ml_dtypes.bfloat16),
            "W2T": np.ascontiguousarray(np.asarray(W2, np.float32).T),
            "b2": np.asarray(b2, np.float32).reshape(1, cfg.C),
            "attl": np.asarray(att_l, np.float32).reshape(1, -1),
            "attr": np.asarray(att_r, np.float32).reshape(1, -1),
            "idxA": wrap16(idxA.astype(np.int16)),
            "normv": normv.astype(ml_dtypes.bfloat16),
            "dinv2": dinv2,
        }
        if TB > 0:
            im["idxB"] = wrap16(idxB.astype(np.int16))
        in_maps.append(im)
    return in_maps, orders, (CA.tolist(), CB.tolist())


def build_nc(cfg: Cfg, CACB):
    CA, CB = (np.asarray(v, dtype=np.int64) for v in CACB)
    TPC, H, C, L = cfg.TPC, cfg.H, cfg.C, cfg.L
    offA = np.zeros(TPC + 1, dtype=np.int64)
    np.cumsum(CA, out=offA[1:])
    offB = np.zeros(TPC + 1, dtype=np.int64)
    np.cumsum(CB, out=offB[1:])
    TA, TB = int(offA[-1]), int(offB[-1])
    NSLC = cfg.F // 128

    nc = bacc.Bacc("TRN2", target_bir_lowering=False, debug=False,
                   num_devices=cfg.M, num_swdge_queues=cfg.QUEUES)
    xT_h = nc.dram_tensor("xT", [cfg.F, cfg.NSHP], BF16, kind="ExternalInput")
    W1T_h = nc.dram_tensor("W1T", [cfg.F, H], BF16, kind="ExternalInput")
    b1_h = nc.dram_tensor("b1", [1, H], BF16, kind="ExternalInput")
    W2T_h = nc.dram_tensor("W2T", [H, C], F32, kind="ExternalInput")
    b2_h = nc.dram_tensor("b2", [1, C], F32, kind="ExternalInput")
    attl_h = nc.dram_tensor("attl", [1, L * H], F32, kind="ExternalInput")
    attr_h = nc.dram_tensor("attr", [1, L * H], F32, kind="ExternalInput")
    idxA_h = nc.dram_tensor("idxA", [128, 8 * TA], I16, kind="ExternalInput")
    if TB > 0:
        idxB_h = nc.dram_tensor("idxB", [128, 8 * TB], I16, kind="ExternalInput")
    normv_h = nc.dram_tensor("normv", [128, TA + TB], BF16, kind="ExternalInput")
    dinv2_h = nc.dram_tensor("dinv2", [128, TPC], F32, kind="ExternalInput")
    out_h = nc.dram_tensor("out", [cfg.NSHP, C], F32, kind="ExternalOutput")

    # chunks: consecutive tiles with both window spans <= CHUNK_COLS
    chunks = []
    t0 = 0
    for t in range(TPC + 1):
        if t == TPC or (t > t0 and
                        (offA[t] - offA[t0] + CA[t] > cfg.CHUNK_COLS or
                         offB[t] - offB[t0] + CB[t] > cfg.CHUNK_COLS)):
            if t > t0:
                chunks.append((t0, t))
            t0 = t
    chunk_of = {}
    for ci, (a, b) in enumerate(chunks):
        for t in range(a, b):
            chunk_of[t] = ci

    with tile.TileContext(nc) as tc:
        with tc.tile_pool(name="dram", bufs=2, space="DRAM") as dram, \
             tc.tile_pool(name="pers", bufs=1) as pers, \
             tc.tile_pool(name="gpool", bufs=4) as gpool, \
             tc.tile_pool(name="cpool", bufs=4) as cpool, \
             tc.tile_pool(name="mpool", bufs=6) as mpool, \
             tc.tile_pool(name="fpool", bufs=2) as fpool, \
             tc.tile_pool(name="nspool", bufs=1) as nspool:

            onesb = pers.tile([1, 128], BF16)
            nc.vector.memset(onesb[:], 1.0)
            ones = pers.tile([1, 128], F32)
            nc.vector.memset(ones[:], 1.0)
            ident = pers.tile([128, 128], F32)
            make_identity(nc, ident[:])
            identb = pers.tile([128, 128], BF16)
            nc.vector.tensor_copy(identb[:], ident[:])
            b1s = pers.tile([1, H], BF16)
            nc.sync.dma_start(b1s[:], b1_h[:])
            b2s = pers.tile([1, C], F32)
            nc.sync.dma_start(b2s[:], b2_h[:])
            W2Ts = pers.tile([H, C], F32)
            nc.sync.dma_start(W2Ts[:], W2T_h[:])
            W1Ts = pers.tile([128, NSLC, H], BF16)
            nc.sync.dma_start(W1Ts[:], W1T_h[:].rearrange("(s p) h -> p s h", p=128))
            attls = pers.tile([1, L * H], F32)
            nc.sync.dma_start(attls[:], attl_h[:])
            attrs = pers.tile([1, L * H], F32)
            nc.sync.dma_start(attrs[:], attr_h[:])
            idxA = pers.tile([128, 8 * TA], I16)
            nc.sync.dma_start(idxA[:], idxA_h[:])
            if TB > 0:
                idxB = pers.tile([128, 8 * TB], I16)
                nc.sync.dma_start(idxB[:], idxB_h[:])
            normv = pers.tile([128, TA + TB], BF16)
            nc.sync.dma_start(normv[:], normv_h[:])
            dinv2 = pers.tile([128, TPC], F32)
            nc.sync.dma_start(dinv2[:], dinv2_h[:])

            attbc = pers.tile([128, max(2 * L, 1), H], F32)
            with tc.tile_pool(name="bpsum", bufs=2, space="PSUM") as bpsum:
                for l in range(L):
                    for j, srcrow in enumerate((attls, attrs)):
                        bc = bpsum.tile([128, H], F32, tag="bc")
                        nc.tensor.matmul(bc[:], lhsT=ones[:],
                                         rhs=srcrow[0:1, l * H:(l + 1) * H],
                                         start=True, stop=True)
                        nc.vector.tensor_copy(attbc[:, 2 * l + j, :], bc[:])

            stage = pers.tile([128, TPC, H], F32)
            raw = pers.tile([128, TPC, H], F32)
            al_sb = pers.tile([128, TPC], F32)
            ar_sb = pers.tile([128, TPC], F32)
            export = pers.tile([128, TPC, H + 1], BF16)
            outs = pers.tile([128, TPC, C], F32)
            mx_all = pers.tile([128, TPC], F32)
            se_all = pers.tile([128, TPC], F32)
            lse_all = pers.tile([128, TPC], F32)

            def node_scores(l):
                """al/ar for layer l, batched: mult-bcast + innermost reduce."""
                for j, dst in ((0, al_sb), (1, ar_sb)):
                    tmp = nspool.tile([128, TPC, H], F32, tag="nsc")
                    nc.vector.tensor_tensor(
                        out=tmp[:], in0=stage[:],
                        in1=attbc[:, 2 * l + j, :].unsqueeze(1).broadcast_to(
                            (128, TPC, H)), op=OP.mult)
                    nc.vector.tensor_reduce(out=dst[:], in_=tmp[:],
                                            axis=mybir.AxisListType.X,
                                            op=OP.add)

            def do_export():
                nc.vector.tensor_copy(export[:, :, 0:H], stage[:])
                nc.vector.tensor_copy(export[:, :, H:H + 1],
                                      al_sb[:].unsqueeze(2))

            # ---- phase 0: h0 = relu(x @ W1.T + b1)
            XG = 4
            with tc.tile_pool(name="xpool", bufs=2) as xpool, \
                 tc.tile_pool(name="ppsum", bufs=4, space="PSUM") as ppsum:
                for g0 in range(0, TPC, XG):
                    g1 = min(g0 + XG, TPC)
                    cw = (g1 - g0) * 128
                    xt = xpool.tile([128, NSLC, cw], BF16, tag="xt")
                    nc.sync.dma_start(
                        xt[:, :, :],
                        xT_h[:, g0 * 128:g1 * 128].rearrange("(s p) c -> p s c", p=128))
                    for t in range(g0, g1):
                        lc = (t - g0) * 128
                        acc = ppsum.tile([128, H], F32, tag="acc")
                        nc.tensor.matmul(acc[:], lhsT=onesb[:], rhs=b1s[:],
                                         start=True, stop=False)
                        for s in range(NSLC):
                            nc.tensor.matmul(acc[:], lhsT=xt[:, s, lc:lc + 128],
                                             rhs=W1Ts[:, s, :],
                                             start=False, stop=(s == NSLC - 1))
                        nc.scalar.activation(stage[:, t, :], acc[:], AF.Relu)
                        nc.vector.tensor_copy(raw[:, t, :], stage[:, t, :])
            node_scores(0)
            do_export()

            # ---- layers
            qctr = 0
            layer_psum = tc.tile_pool(name="qpsum", bufs=3, space="PSUM")
            qpsum = layer_psum.__enter__()
            for l in range(L):
                tbl_in = dram.tile([cfg.NSHP, 128], BF16, tag="tbl_in")
                tbl_full = dram.tile([cfg.RF, 128], BF16, tag="tbl_full",
                                     addr_space="Shared")
                nc.sync.dma_start(
                    tbl_in[:].rearrange("(t p) e -> p t e", p=128)[:, :, 0:H + 1],
                    export[:])
                nc.gpsimd.collective_compute(
                    "AllGather", OP.bypass,
                    replica_groups=[list(range(cfg.M))],
                    ins=[tbl_in.opt()], outs=[tbl_full.opt()])

                # self-loop coefficient per node
                cs = cpool.tile([128, TPC], F32, tag="cs")
                nc.vector.tensor_tensor(out=cs[:], in0=al_sb[:], in1=ar_sb[:],
                                        op=OP.add)
                nc.scalar.activation(cs[:], cs[:], AF.Tanh)
                nc.vector.tensor_tensor(out=cs[:], in0=cs[:], in1=dinv2[:],
                                        op=OP.mult)

                cur_chunk = -1
                gA = gB = msgA = msgB = None
                po = None
                g_t0 = 0

                def open_chunk(ci):
                    nonlocal gA, gB, msgA, msgB, qctr
                    ct0, ct1 = chunks[ci]
                    cA0, cA1 = int(offA[ct0]), int(offA[ct1])
                    cB0, cB1 = int(offB[ct0]), int(offB[ct1])
                    spanA, spanB = cA1 - cA0, cB1 - cB0
                    gA = gpool.tile([128, cfg.CHUNK_COLS, 128], BF16, tag="gA")
                    nc.gpsimd.dma_gather(
                        out_ap=gA[:, :spanA, :],
                        in_ap=tbl_full[:cfg.WINDOW, :],
                        idxs_ap=idxA[:, 8 * cA0:8 * cA1],
                        num_idxs=128 * spanA, num_idxs_reg=128 * spanA,
                        elem_size=128, single_packet=False,
                        queue_num=qctr % cfg.QUEUES)
                    qctr += 1
                    if TB > 0 and spanB > 0:
                        gB = gpool.tile([128, cfg.CHUNK_COLS, 128], BF16,
                                        tag="gB")
                        nc.gpsimd.dma_gather(
                            out_ap=gB[:, :spanB, :],
                            in_ap=tbl_full[cfg.RF - cfg.WINDOW:, :],
                            idxs_ap=idxB[:, 8 * cB0:8 * cB1],
                            num_idxs=128 * spanB, num_idxs_reg=128 * spanB,
                            elem_size=128, single_packet=False,
                            queue_num=qctr % cfg.QUEUES)
                        qctr += 1
                    # coeff: tanh(al_src + ar_dst) per tile, then * norm batched
                    cfA = cpool.tile([128, cfg.CHUNK_COLS], BF16, tag="cfA")
                    cfB = cpool.tile([128, cfg.CHUNK_COLS], BF16, tag="cfB")
                    for t in range(ct0, ct1):
                        nA = int(CA[t])
                        lcA = int(offA[t]) - cA0
                        nc.scalar.activation(cfA[:, lcA:lcA + nA],
                                             gA[:, lcA:lcA + nA, H],
                                             AF.Tanh, bias=ar_sb[:, t:t + 1])
                        nB = int(CB[t])
                        if nB > 0:
                            lcB = int(offB[t]) - cB0
                            nc.scalar.activation(cfB[:, lcB:lcB + nB],
                                                 gB[:, lcB:lcB + nB, H],
                                                 AF.Tanh, bias=ar_sb[:, t:t + 1])
                    nc.vector.tensor_tensor(
                        out=cfA[:, :spanA], in0=cfA[:, :spanA],
                        in1=normv[:, cA0:cA1], op=OP.mult)
                    msgA = mpool.tile([128, cfg.CHUNK_COLS, H], BF16, tag="mA")
                    nc.vector.tensor_tensor(
                        out=msgA[:, :spanA, :], in0=gA[:, :spanA, 0:H],
                        in1=cfA[:, :spanA].unsqueeze(2).broadcast_to(
                            (128, spanA, H)), op=OP.mult)
                    if spanB > 0:
                        nc.vector.tensor_tensor(
                            out=cfB[:, :spanB], in0=cfB[:, :spanB],
                            in1=normv[:, TA + cB0:TA + cB1], op=OP.mult)
                        msgB = mpool.tile([128, cfg.CHUNK_COLS, H], BF16,
                                          tag="mB")
                        nc.vector.tensor_tensor(
                            out=msgB[:, :spanB, :], in0=gB[:, :spanB, 0:H],
                            in1=cfB[:, :spanB].unsqueeze(2).broadcast_to(
                                (128, spanB, H)), op=OP.mult)

                def fold_group(t0g, t1g):
                    n = t1g - t0g
                    # one PSUM operand per DVE op: copy, then accumulate
                    c0 = fpool.tile([128, cfg.GT, 128], F32, tag="c0")
                    nc.vector.tensor_copy(c0[:, :n, :],
                                          po[:, t0g - g_t0:t1g - g_t0, 0:128])
                    nc.vector.tensor_tensor(out=c0[:, :n, :],
                                            in0=c0[:, :n, :],
                                            in1=po[:, t0g - g_t0:t1g - g_t0, 128:256],
                                            op=OP.add)
                    f3 = fpool.tile([128, cfg.GT, H], F32, tag="f3")
                    nc.vector.tensor_tensor(out=f3[:, :n, :],
                                            in0=c0[:, :n, 0:H],
                                            in1=c0[:, :n, H:128], op=OP.add)
                    ms = fpool.tile([128, cfg.GT, H], F32, tag="ms")
                    nc.vector.tensor_tensor(
                        out=ms[:, :n, :], in0=stage[:, t0g:t1g, :],
                        in1=cs[:, t0g:t1g].unsqueeze(2).broadcast_to(
                            (128, n, H)), op=OP.mult)
                    nc.vector.scalar_tensor_tensor(
                        out=stage[:, t0g:t1g, :], in0=raw[:, t0g:t1g, :],
                        scalar=cfg.EPS, in1=f3[:, :n, :],
                        op0=OP.mult, op1=OP.add)
                    nc.vector.tensor_tensor(out=stage[:, t0g:t1g, :],
                                            in0=stage[:, t0g:t1g, :],
                                            in1=ms[:, :n, :], op=OP.add)

                for t in range(TPC):
                    ci = chunk_of[t]
                    if ci != cur_chunk:
                        open_chunk(ci)
                        cur_chunk = ci
                    if t % cfg.GT == 0:
                        g_t0 = t
                        po = qpsum.tile([128, cfg.GT, 256], F32, tag="po")
                    j = t - g_t0
                    ct0 = chunks[ci][0]
                    lcA = int(offA[t]) - int(offA[ct0])
                    lcB = int(offB[t]) - int(offB[ct0])
                    nA, nB = int(CA[t]), int(CB[t])
                    W = 4  # slots per wide matmul (256 psum cols)
                    ops = []  # (msg, lc, nslots) full groups first, then ragged
                    for g in range(nA // W):
                        ops.append((msgA, lcA + W * g, W))
                    for g in range(nB // W):
                        ops.append((msgB, lcB + W * g, W))
                    if nA % W:
                        ops.append((msgA, lcA + W * (nA // W), nA % W))
                    if nB % W:
                        ops.append((msgB, lcB + W * (nB // W), nB % W))
                    assert ops and ops[0][2] == W
                    for oi, (mm, lc, ns) in enumerate(ops):
                        cols = 64 * ns
                        nc.tensor.matmul(
                            po[:, j, 256 - cols:256],
                            lhsT=identb[:],
                            rhs=mm[:, lc:lc + ns, :],
                            start=(oi == 0), stop=(oi == len(ops) - 1),
                            skip_group_check=True)
                    if t % cfg.GT == cfg.GT - 1 or t == TPC - 1:
                        fold_group(g_t0, t + 1)

                if l < L - 1:
                    node_scores(l + 1)
                    do_export()
            layer_psum.__exit__(None, None, None)

            # ---- logits + log_softmax
            with tc.tile_pool(name="spool", bufs=2) as spool, \
                 tc.tile_pool(name="lpsum", bufs=4, space="PSUM") as lpsum:
                for t in range(TPC):
                    tr = lpsum.tile([H, 128], F32, tag="tr")
                    nc.tensor.transpose(out=tr[:], in_=stage[:, t, :],
                                        identity=ident[:])
                    htT = spool.tile([H, 128], F32, tag="htT")
                    nc.vector.tensor_copy(htT[:], tr[:])
                    lg = lpsum.tile([128, C], F32, tag="lg")
                    nc.tensor.matmul(lg[:], lhsT=ones[:], rhs=b2s[:],
                                     start=True, stop=False)
                    nc.tensor.matmul(lg[:], lhsT=htT[:], rhs=W2Ts[:],
                                     start=False, stop=True)
                    nc.vector.tensor_reduce(out=mx_all[:, t:t + 1], in_=lg[:],
                                            axis=mybir.AxisListType.X, op=OP.max,
                                            negate=True)
                    scr40 = cpool.tile([128, C], F32, tag="scr40")
                    nc.scalar.activation(scr40[:], lg[:], AF.Exp,
                                         bias=mx_all[:, t:t + 1],
                                         accum_out=se_all[:, t:t + 1])
                    nc.vector.tensor_copy(outs[:, t, :], lg[:])
                nc.scalar.activation(lse_all[:], se_all[:], AF.Ln)
                for t in range(TPC):
                    nc.vector.tensor_scalar(
                        out=outs[:, t, :], in0=outs[:, t, :],
                        scalar1=mx_all[:, t:t + 1], scalar2=lse_all[:, t:t + 1],
                        op0=OP.add, op1=OP.subtract)
                nc.sync.dma_start(out_h[:].rearrange("(t p) c -> p t c", p=128),
                                  outs[:])
    nc.compile()
    return nc


def run(cfg: Cfg, inputs: dict, trace: bool = False):
    in_maps, orders, CACB = host_prep(cfg, **inputs)
    nc = build_nc(cfg, CACB)
    res = bass_utils.run_bass_kernel_spmd(
        nc, in_maps, core_ids=list(range(cfg.M)), trace=trace)
    out = np.empty((cfg.N, cfg.C), dtype=np.float32)
    for k in range(cfg.M):
        out[k * cfg.NSH + orders[k]] = np.asarray(res.results[k]["out"],
                                                  np.float32)[:cfg.NSH]
    return out, res


def kernel(x, edge_index, W1, b1, W2, b2, att_l, att_r):
    cfg = Cfg()
    out, _ = run(cfg, dict(x=np.asarray(x, np.float32),
                           edge_index=np.asarray(edge_index),
                           W1=W1, b1=b1, W2=W2, b2=b2,
                           att_l=att_l, att_r=att_r))
    return out


# revision 16
# speedup vs baseline: 3.6594x; 2.1440x over previous
"""FAGCN (4-layer FAConv + lin1/lin2 + log_softmax) on 8 Trainium2 cores.

v2 — gather-descriptor-bound baseline reworked:
- bf16 h-table rows of 256B (h(64)|al|junk) halve gather + AllGather bytes.
- dma_gather round-robin over 4 SWDGE queues: desc-gen runs on Q7 core pair
  (2q, 2q+1), so 4 queues ~4x the descriptor throughput (was the bottleneck:
  78% GpSimd busy on queue 0 only).
- Self-loop messages computed on-chip (per-node, batched DVE) instead of
  gathered; removes 6.25k slots/core and shrinks per-node slot counts.
- (d, n2-n0) node ordering lowers shared CA/CB slot padding 1.48x -> 1.36x.
- Per-slot DVE scaling replaced by per-chunk broadcast (stride-0 AP) multiply;
  coeff tanh stays per-tile on ACT (ar as per-partition bias).
- Per-slot identity matmuls replaced by 8-slot-wide matmuls into a [128,512]
  PSUM bank per tile (slot k -> col block k%8; blocks summed by a batched
  DVE tree fold per 4-tile group).
- Phase 0 (x @ W1) in bf16.
"""
import numpy as np
from dataclasses import dataclass

import concourse.bass as bass
import concourse.bacc as bacc
import concourse.tile as tile
import concourse.mybir as mybir
from concourse import bass_utils
from concourse.masks import make_identity

F32 = mybir.dt.float32
BF16 = mybir.dt.bfloat16
I16 = mybir.dt.int16
AF = mybir.ActivationFunctionType
OP = mybir.AluOpType


@dataclass
class Cfg:
    N: int = 50000
    E: int = 800000
    F: int = 512
    H: int = 64
    C: int = 40
    L: int = 4
    EPS: float = 0.2
    M: int = 8           # cores
    CHUNK_COLS: int = 32
    WINDOW: int = 32768  # dma_gather int16 index limit
    QUEUES: int = 4      # SWDGE queues for gather desc-gen
    GT: int = 4          # tiles per psum fold group

    @property
    def NSH(self):
        return self.N // self.M

    @property
    def TPC(self):
        return (self.NSH + 127) // 128

    @property
    def NSHP(self):
        return self.TPC * 128

    @property
    def RF(self):
        return self.NSHP * self.M


def host_prep(cfg: Cfg, x, edge_index, W1, b1, W2, b2, att_l, att_r):
    """Shard + order nodes, build window-split gather/norm arrays (no loops)."""
    import ml_dtypes
    N, M, NSH, NSHP, TPC = cfg.N, cfg.M, cfg.NSH, cfg.NSHP, cfg.TPC
    src = np.asarray(edge_index[0], dtype=np.int64)
    dst = np.asarray(edge_index[1], dtype=np.int64)
    deg = np.bincount(dst, minlength=N).astype(np.float32) + 1.0  # + self loop
    dinv = (1.0 / np.sqrt(deg)).astype(np.float32)
    norm_e = (dinv[src] * dinv[dst]).astype(np.float32)
    core_of = dst // NSH
    B_BASE = cfg.RF - cfg.WINDOW

    per_core = []  # (es, dl, n0_, n1_)? computed per iteration
    for k in range(M):
        m = core_of == k
        per_core.append((src[m], dst[m] - k * NSH, norm_e[m]))

    def feats(grow_map):
        out = []
        for k in range(M):
            es, ds, _ = per_core[k]
            grow = grow_map[es]
            cls = np.where(grow >= cfg.WINDOW, 2,
                           np.where(grow >= B_BASE, 1, 0))
            n0 = np.bincount(ds[cls == 0], minlength=NSH)
            n2 = np.bincount(ds[cls == 2], minlength=NSH)
            d = np.bincount(ds, minlength=NSH)
            out.append((n0, n2, d))
        return out

    def grow_from(orders):
        gm = np.empty(N, dtype=np.int64)
        for k in range(M):
            inv = np.empty(NSH, dtype=np.int64)
            inv[orders[k]] = np.arange(NSH)
            gm[k * NSH:(k + 1) * NSH] = k * NSHP + inv
        return gm

    orders = [np.arange(NSH) for _ in range(M)]
    for _ in range(2):
        f = feats(grow_from(orders))
        orders = [np.lexsort((f[k][1] - f[k][0], -f[k][2])) for k in range(M)]
    grow_map = grow_from(orders)
    f = feats(grow_map)

    # shared per-tile window budgets
    A0 = np.zeros(TPC, dtype=np.int64)
    B2 = np.zeros(TPC, dtype=np.int64)
    D = np.zeros(TPC, dtype=np.int64)
    inv_orders = []
    for k in range(M):
        inv = np.empty(NSH, dtype=np.int64)
        inv[orders[k]] = np.arange(NSH)
        inv_orders.append(inv)
        n0o = np.zeros(NSHP, dtype=np.int64)
        n2o = np.zeros(NSHP, dtype=np.int64)
        do = np.zeros(NSHP, dtype=np.int64)
        n0o[:NSH] = f[k][0][orders[k]]
        n2o[:NSH] = f[k][1][orders[k]]
        do[:NSH] = f[k][2][orders[k]]
        A0 = np.maximum(A0, n0o.reshape(TPC, 128).max(1))
        B2 = np.maximum(B2, n2o.reshape(TPC, 128).max(1))
        D = np.maximum(D, do.reshape(TPC, 128).max(1))
    cost = np.maximum(A0 + B2, np.maximum(D, 4))
    CA = np.maximum(np.maximum(A0, cost - B2), 4)  # >=1 full 4-slot A group
    CB = np.maximum(cost - CA, B2)
    assert CA.max() <= cfg.CHUNK_COLS and CB.max() <= cfg.CHUNK_COLS, \
        (CA.max(), CB.max())
    offA = np.zeros(TPC + 1, dtype=np.int64)
    np.cumsum(CA, out=offA[1:])
    offB = np.zeros(TPC + 1, dtype=np.int64)
    np.cumsum(CB, out=offB[1:])
    TA, TB = int(offA[-1]), int(offB[-1])

    def wrap16(lst16):
        a = lst16.reshape(-1, 16).T.copy()
        return np.tile(a, (8, 1)).astype(np.int16)

    perm_f = None  # no feature permutation
    in_maps = []
    for k in range(M):
        es, ds, en = per_core[k]
        rk = inv_orders[k][ds]
        grow = grow_map[es]
        cls = np.where(grow >= cfg.WINDOW, 2,
                       np.where(grow >= B_BASE, 1, 0)).astype(np.int8)
        n0 = np.bincount(rk[cls == 0], minlength=NSHP)
        n1 = np.bincount(rk[cls == 1], minlength=NSHP)
        d = np.bincount(rk, minlength=NSHP)
        t_all0 = np.arange(NSHP) // 128
        nlo = np.minimum(CA[t_all0][np.arange(NSHP)], n0 + n1)
        nlo = np.maximum(nlo, n0)
        assert (d - nlo <= CB[t_all0]).all()

        o = np.lexsort((cls, rk))
        rk_s, en_s, grow_s, cls_s = rk[o], en[o], grow[o], cls[o]
        dl = np.bincount(rk_s, minlength=NSHP)
        run0 = np.repeat(np.cumsum(np.concatenate([[0], dl]))[:-1], dl)
        j = np.arange(len(rk_s)) - run0
        is_lo = j < nlo[rk_s]
        p_all = rk_s % 128
        t_all = rk_s // 128
        colA = offA[t_all] + j
        colB = offB[t_all] + (j - nlo[rk_s])
        posA = colA[is_lo] * 128 + p_all[is_lo]
        posB = colB[~is_lo] * 128 + p_all[~is_lo]

        # padding slots get SPREAD indices (not 0): thousands of concurrent
        # reads of one row serialize at the HBM bank and stall the drain.
        idxA = (np.arange(TA * 128, dtype=np.int64) * 2654435761) % cfg.WINDOW
        idxA[posA] = grow_s[is_lo]
        normv = np.zeros((128, TA + TB), dtype=np.float32)
        normv[p_all[is_lo], colA[is_lo]] = en_s[is_lo]
        assert idxA.max() < cfg.WINDOW
        if TB > 0:
            idxB = (np.arange(TB * 128, dtype=np.int64) * 2654435761) % cfg.WINDOW
            idxB[posB] = grow_s[~is_lo] - B_BASE
            normv[p_all[~is_lo], TA + colB[~is_lo]] = en_s[~is_lo]
            assert idxB.min() >= 0 and idxB.max() < cfg.WINDOW

        xk = np.zeros((cfg.F, NSHP), dtype=ml_dtypes.bfloat16)
        xk[:, :NSH] = np.asarray(x[k * NSH:(k + 1) * NSH],
                                 np.float32)[orders[k]].T.astype(
                                     ml_dtypes.bfloat16)
        dinv2 = np.zeros((128, TPC), dtype=np.float32)
        dk = dinv[k * NSH:(k + 1) * NSH][orders[k]] ** 2
        dinv2[:, :] = np.pad(dk, (0, NSHP - NSH)).reshape(TPC, 128).T

        im = {
            "xT": np.ascontiguousarray(xk),
            "W1T": np.ascontiguousarray(
                np.asarray(W1, np.float32).T.astype(ml_dtypes.bfloat16)),
            "b1": np.asarray(b1, np.float32).reshape(1, cfg.H).astype(
                ml_dtypes.bfloat16),
            "W2T": np.ascontiguousarray(np.asarray(W2, np.float32).T),
            "b2": np.asarray(b2, np.float32).reshape(1, cfg.C),
            "attl": np.asarray(att_l, np.float32).reshape(1, -1),
            "attr": np.asarray(att_r, np.float32).reshape(1, -1),
            "idxA": wrap16(idxA.astype(np.int16)),
            "normv": normv.astype(ml_dtypes.bfloat16),
            "dinv2": dinv2,
        }
        if TB > 0:
            im["idxB"] = wrap16(idxB.astype(np.int16))
        in_maps.append(im)
    return in_maps, orders, (CA.tolist(), CB.tolist())


def build_nc(cfg: Cfg, CACB):
    CA, CB = (np.asarray(v, dtype=np.int64) for v in CACB)
    TPC, H, C, L = cfg.TPC, cfg.H, cfg.C, cfg.L
    offA = np.zeros(TPC + 1, dtype=np.int64)
    np.cumsum(CA, out=offA[1:])
    offB = np.zeros(TPC + 1, dtype=np.int64)
    np.cumsum(CB, out=offB[1:])
    TA, TB = int(offA[-1]), int(offB[-1])
    NSLC = cfg.F // 128

    nc = bacc.Bacc("TRN2", target_bir_lowering=False, debug=False,
                   num_devices=cfg.M, num_swdge_queues=cfg.QUEUES)
    xT_h = nc.dram_tensor("xT", [cfg.F, cfg.NSHP], BF16, kind="ExternalInput")
    W1T_h = nc.dram_tensor("W1T", [cfg.F, H], BF16, kind="ExternalInput")
    b1_h = nc.dram_tensor("b1", [1, H], BF16, kind="ExternalInput")
    W2T_h = nc.dram_tensor("W2T", [H, C], F32, kind="ExternalInput")
    b2_h = nc.dram_tensor("b2", [1, C], F32, kind="ExternalInput")
    attl_h = nc.dram_tensor("attl", [1, L * H], F32, kind="ExternalInput")
    attr_h = nc.dram_tensor("attr", [1, L * H], F32, kind="ExternalInput")
    idxA_h = nc.dram_tensor("idxA", [128, 8 * TA], I16, kind="ExternalInput")
    if TB > 0:
        idxB_h = nc.dram_tensor("idxB", [128, 8 * TB], I16, kind="ExternalInput")
    normv_h = nc.dram_tensor("normv", [128, TA + TB], BF16, kind="ExternalInput")
    dinv2_h = nc.dram_tensor("dinv2", [128, TPC], F32, kind="ExternalInput")
    out_h = nc.dram_tensor("out", [cfg.NSHP, C], F32, kind="ExternalOutput")

    # chunks: consecutive tiles with both window spans <= CHUNK_COLS
    chunks = []
    t0 = 0
    for t in range(TPC + 1):
        if t == TPC or (t > t0 and
                        (offA[t] - offA[t0] + CA[t] > cfg.CHUNK_COLS or
                         offB[t] - offB[t0] + CB[t] > cfg.CHUNK_COLS)):
            if t > t0:
                chunks.append((t0, t))
            t0 = t
    chunk_of = {}
    for ci, (a, b) in enumerate(chunks):
        for t in range(a, b):
            chunk_of[t] = ci

    with tile.TileContext(nc) as tc:
        with tc.tile_pool(name="dram", bufs=2, space="DRAM") as dram, \
             tc.tile_pool(name="pers", bufs=1) as pers, \
             tc.tile_pool(name="gpool", bufs=4) as gpool, \
             tc.tile_pool(name="cpool", bufs=4) as cpool, \
             tc.tile_pool(name="mpool", bufs=6) as mpool, \
             tc.tile_pool(name="fpool", bufs=2) as fpool, \
             tc.tile_pool(name="nspool", bufs=1) as nspool:

            onesb = pers.tile([1, 128], BF16)
            nc.vector.memset(onesb[:], 1.0)
            ones = pers.tile([1, 128], F32)
            nc.vector.memset(ones[:], 1.0)
            ident = pers.tile([128, 128], F32)
            make_identity(nc, ident[:])
            identb = pers.tile([128, 128], BF16)
            nc.vector.tensor_copy(identb[:], ident[:])
            b1s = pers.tile([1, H], BF16)
            nc.sync.dma_start(b1s[:], b1_h[:])
            b2s = pers.tile([1, C], F32)
            nc.sync.dma_start(b2s[:], b2_h[:])
            W2Ts = pers.tile([H, C], F32)
            nc.sync.dma_start(W2Ts[:], W2T_h[:])
            W1Ts = pers.tile([128, NSLC, H], BF16)
            nc.sync.dma_start(W1Ts[:], W1T_h[:].rearrange("(s p) h -> p s h", p=128))
            attls = pers.tile([1, L * H], F32)
            nc.sync.dma_start(attls[:], attl_h[:])
            attrs = pers.tile([1, L * H], F32)
            nc.sync.dma_start(attrs[:], attr_h[:])
            idxA = pers.tile([128, 8 * TA], I16)
            nc.sync.dma_start(idxA[:], idxA_h[:])
            if TB > 0:
                idxB = pers.tile([128, 8 * TB], I16)
                nc.sync.dma_start(idxB[:], idxB_h[:])
            normv = pers.tile([128, TA + TB], BF16)
            nc.sync.dma_start(normv[:], normv_h[:])
            dinv2 = pers.tile([128, TPC], F32)
            nc.sync.dma_start(dinv2[:], dinv2_h[:])

            attbc = pers.tile([128, max(2 * L, 1), H], F32)
            with tc.tile_pool(name="bpsum", bufs=2, space="PSUM") as bpsum:
                for l in range(L):
                    for j, srcrow in enumerate((attls, attrs)):
                        bc = bpsum.tile([128, H], F32, tag="bc")
                        nc.tensor.matmul(bc[:], lhsT=ones[:],
                                         rhs=srcrow[0:1, l * H:(l + 1) * H],
                                         start=True, stop=True)
                        nc.vector.tensor_copy(attbc[:, 2 * l + j, :], bc[:])

            stage = pers.tile([128, TPC, H], F32)
            raw = pers.tile([128, TPC, H], F32)
            al_sb = pers.tile([128, TPC], F32)
            ar_sb = pers.tile([128, TPC], F32)
            export = pers.tile([128, TPC, H + 1], BF16)
            outs = pers.tile([128, TPC, C], F32)
            mx_all = pers.tile([128, TPC], F32)
            se_all = pers.tile([128, TPC], F32)
            lse_all = pers.tile([128, TPC], F32)

            def node_scores(l):
                """al/ar for layer l, batched: mult-bcast + innermost reduce."""
                for j, dst in ((0, al_sb), (1, ar_sb)):
                    tmp = nspool.tile([128, TPC, H], F32, tag="nsc")
                    nc.vector.tensor_tensor(
                        out=tmp[:], in0=stage[:],
                        in1=attbc[:, 2 * l + j, :].unsqueeze(1).broadcast_to(
                            (128, TPC, H)), op=OP.mult)
                    nc.vector.tensor_reduce(out=dst[:], in_=tmp[:],
                                            axis=mybir.AxisListType.X,
                                            op=OP.add)

            def do_export():
                nc.vector.tensor_copy(export[:, :, 0:H], stage[:])
                nc.vector.tensor_copy(export[:, :, H:H + 1],
                                      al_sb[:].unsqueeze(2))

            # ---- phase 0: h0 = relu(x @ W1.T + b1)
            XG = 4
            with tc.tile_pool(name="xpool", bufs=2) as xpool, \
                 tc.tile_pool(name="ppsum", bufs=4, space="PSUM") as ppsum:
                for g0 in range(0, TPC, XG):
                    g1 = min(g0 + XG, TPC)
                    cw = (g1 - g0) * 128
                    xt = xpool.tile([128, NSLC, cw], BF16, tag="xt")
                    nc.sync.dma_start(
                        xt[:, :, :],
                        xT_h[:, g0 * 128:g1 * 128].rearrange("(s p) c -> p s c", p=128))
                    for t in range(g0, g1):
                        lc = (t - g0) * 128
                        acc = ppsum.tile([128, H], F32, tag="acc")
                        nc.tensor.matmul(acc[:], lhsT=onesb[:], rhs=b1s[:],
                                         start=True, stop=False)
                        for s in range(NSLC):
                            nc.tensor.matmul(acc[:], lhsT=xt[:, s, lc:lc + 128],
                                             rhs=W1Ts[:, s, :],
                                             start=False, stop=(s == NSLC - 1))
                        nc.scalar.activation(stage[:, t, :], acc[:], AF.Relu)
                        nc.vector.tensor_copy(raw[:, t, :], stage[:, t, :])
            node_scores(0)
            do_export()

            # ---- layers
            qctr = 0
            layer_psum = tc.tile_pool(name="qpsum", bufs=3, space="PSUM")
            qpsum = layer_psum.__enter__()
            for l in range(L):
                tbl_in = dram.tile([cfg.NSHP, 128], BF16, tag="tbl_in")
                tbl_full = dram.tile([cfg.RF, 128], BF16, tag="tbl_full",
                                     addr_space="Shared")
                nc.sync.dma_start(
                    tbl_in[:].rearrange("(t p) e -> p t e", p=128)[:, :, 0:H + 1],
                    export[:])
                nc.gpsimd.collective_compute(
                    "AllGather", OP.bypass,
                    replica_groups=[list(range(cfg.M))],
                    ins=[tbl_in.opt()], outs=[tbl_full.opt()])

                # self-loop coefficient per node
                cs = cpool.tile([128, TPC], F32, tag="cs")
                nc.vector.tensor_tensor(out=cs[:], in0=al_sb[:], in1=ar_sb[:],
                                        op=OP.add)
                nc.scalar.activation(cs[:], cs[:], AF.Tanh)
                nc.vector.tensor_tensor(out=cs[:], in0=cs[:], in1=dinv2[:],
                                        op=OP.mult)

                import os
                TRIVIAL = os.environ.get("KPERF_TRIVIAL") == "1"
                cur_chunk = -1
                gA = gB = msgA = msgB = None
                po = None
                g_t0 = 0

                def open_chunk(ci):
                    nonlocal gA, gB, msgA, msgB, qctr
                    ct0, ct1 = chunks[ci]
                    cA0, cA1 = int(offA[ct0]), int(offA[ct1])
                    cB0, cB1 = int(offB[ct0]), int(offB[ct1])
                    spanA, spanB = cA1 - cA0, cB1 - cB0
                    gA = gpool.tile([128, cfg.CHUNK_COLS, 128], BF16, tag="gA")
                    nc.gpsimd.dma_gather(
                        out_ap=gA[:, :spanA, :],
                        in_ap=tbl_full[:cfg.WINDOW, :],
                        idxs_ap=idxA[:, 8 * cA0:8 * cA1],
                        num_idxs=128 * spanA, num_idxs_reg=128 * spanA,
                        elem_size=128, single_packet=False,
                        queue_num=qctr % cfg.QUEUES)
                    qctr += 1
                    if TB > 0 and spanB > 0:
                        gB = gpool.tile([128, cfg.CHUNK_COLS, 128], BF16,
                                        tag="gB")
                        nc.gpsimd.dma_gather(
                            out_ap=gB[:, :spanB, :],
                            in_ap=tbl_full[cfg.RF - cfg.WINDOW:, :],
                            idxs_ap=idxB[:, 8 * cB0:8 * cB1],
                            num_idxs=128 * spanB, num_idxs_reg=128 * spanB,
                            elem_size=128, single_packet=False,
                            queue_num=qctr % cfg.QUEUES)
                        qctr += 1
                    if TRIVIAL:
                        nc.vector.tensor_tensor(out=trivacc[:], in0=trivacc[:],
                                                in1=gA[:, 0, 0:1], op=OP.add)
                        if spanB > 0:
                            nc.vector.tensor_tensor(out=trivacc[:],
                                                    in0=trivacc[:],
                                                    in1=gB[:, 0, 0:1], op=OP.add)
                        return
                    # coeff: tanh(al_src + ar_dst) per tile, then * norm batched
                    cfA = cpool.tile([128, cfg.CHUNK_COLS], BF16, tag="cfA")
                    cfB = cpool.tile([128, cfg.CHUNK_COLS], BF16, tag="cfB")
                    for t in range(ct0, ct1):
                        nA = int(CA[t])
                        lcA = int(offA[t]) - cA0
                        nc.scalar.activation(cfA[:, lcA:lcA + nA],
                                             gA[:, lcA:lcA + nA, H],
                                             AF.Tanh, bias=ar_sb[:, t:t + 1])
                        nB = int(CB[t])
                        if nB > 0:
                            lcB = int(offB[t]) - cB0
                            nc.scalar.activation(cfB[:, lcB:lcB + nB],
                                                 gB[:, lcB:lcB + nB, H],
                                                 AF.Tanh, bias=ar_sb[:, t:t + 1])
                    nc.vector.tensor_tensor(
                        out=cfA[:, :spanA], in0=cfA[:, :spanA],
                        in1=normv[:, cA0:cA1], op=OP.mult)
                    msgA = mpool.tile([128, cfg.CHUNK_COLS, H], BF16, tag="mA")
                    nc.vector.tensor_tensor(
                        out=msgA[:, :spanA, :], in0=gA[:, :spanA, 0:H],
                        in1=cfA[:, :spanA].unsqueeze(2).broadcast_to(
                            (128, spanA, H)), op=OP.mult)
                    if spanB > 0:
                        nc.vector.tensor_tensor(
                            out=cfB[:, :spanB], in0=cfB[:, :spanB],
                            in1=normv[:, TA + cB0:TA + cB1], op=OP.mult)
                        msgB = mpool.tile([128, cfg.CHUNK_COLS, H], BF16,
                                          tag="mB")
                        nc.vector.tensor_tensor(
                            out=msgB[:, :spanB, :], in0=gB[:, :spanB, 0:H],
                            in1=cfB[:, :spanB].unsqueeze(2).broadcast_to(
                                (128, spanB, H)), op=OP.mult)

                def fold_group(t0g, t1g):
                    n = t1g - t0g
                    # one PSUM operand per DVE op: copy, then accumulate
                    c0 = fpool.tile([128, cfg.GT, 128], F32, tag="c0")
                    nc.vector.tensor_copy(c0[:, :n, :],
                                          po[:, t0g - g_t0:t1g - g_t0, 0:128])
                    nc.vector.tensor_tensor(out=c0[:, :n, :],
                                            in0=c0[:, :n, :],
                                            in1=po[:, t0g - g_t0:t1g - g_t0, 128:256],
                                            op=OP.add)
                    f3 = fpool.tile([128, cfg.GT, H], F32, tag="f3")
                    nc.vector.tensor_tensor(out=f3[:, :n, :],
                                            in0=c0[:, :n, 0:H],
                                            in1=c0[:, :n, H:128], op=OP.add)
                    ms = fpool.tile([128, cfg.GT, H], F32, tag="ms")
                    nc.vector.tensor_tensor(
                        out=ms[:, :n, :], in0=stage[:, t0g:t1g, :],
                        in1=cs[:, t0g:t1g].unsqueeze(2).broadcast_to(
                            (128, n, H)), op=OP.mult)
                    nc.vector.scalar_tensor_tensor(
                        out=stage[:, t0g:t1g, :], in0=raw[:, t0g:t1g, :],
                        scalar=cfg.EPS, in1=f3[:, :n, :],
                        op0=OP.mult, op1=OP.add)
                    nc.vector.tensor_tensor(out=stage[:, t0g:t1g, :],
                                            in0=stage[:, t0g:t1g, :],
                                            in1=ms[:, :n, :], op=OP.add)

                trivacc = cpool.tile([128, 1], F32, tag="triv")
                if TRIVIAL:
                    nc.vector.memset(trivacc[:], 0.0)
                for t in range(TPC):
                    ci = chunk_of[t]
                    if ci != cur_chunk:
                        open_chunk(ci)
                        cur_chunk = ci
                    if TRIVIAL:
                        continue
                    if t % cfg.GT == 0:
                        g_t0 = t
                        po = qpsum.tile([128, cfg.GT, 256], F32, tag="po")
                    j = t - g_t0
                    ct0 = chunks[ci][0]
                    lcA = int(offA[t]) - int(offA[ct0])
                    lcB = int(offB[t]) - int(offB[ct0])
                    nA, nB = int(CA[t]), int(CB[t])
                    W = 4  # slots per wide matmul (256 psum cols)
                    ops = []  # (msg, lc, nslots) full groups first, then ragged
                    for g in range(nA // W):
                        ops.append((msgA, lcA + W * g, W))
                    for g in range(nB // W):
                        ops.append((msgB, lcB + W * g, W))
                    if nA % W:
                        ops.append((msgA, lcA + W * (nA // W), nA % W))
                    if nB % W:
                        ops.append((msgB, lcB + W * (nB // W), nB % W))
                    assert ops and ops[0][2] == W
                    for oi, (mm, lc, ns) in enumerate(ops):
                        cols = 64 * ns
                        nc.tensor.matmul(
                            po[:, j, 256 - cols:256],
                            lhsT=identb[:],
                            rhs=mm[:, lc:lc + ns, :],
                            start=(oi == 0), stop=(oi == len(ops) - 1),
                            skip_group_check=True)
                    if t % cfg.GT == cfg.GT - 1 or t == TPC - 1:
                        fold_group(g_t0, t + 1)

                if l < L - 1:
                    node_scores(l + 1)
                    do_export()
            layer_psum.__exit__(None, None, None)

            # ---- logits + log_softmax
            with tc.tile_pool(name="spool", bufs=2) as spool, \
                 tc.tile_pool(name="lpsum", bufs=4, space="PSUM") as lpsum:
                for t in range(TPC):
                    tr = lpsum.tile([H, 128], F32, tag="tr")
                    nc.tensor.transpose(out=tr[:], in_=stage[:, t, :],
                                        identity=ident[:])
                    htT = spool.tile([H, 128], F32, tag="htT")
                    nc.vector.tensor_copy(htT[:], tr[:])
                    lg = lpsum.tile([128, C], F32, tag="lg")
                    nc.tensor.matmul(lg[:], lhsT=ones[:], rhs=b2s[:],
                                     start=True, stop=False)
                    nc.tensor.matmul(lg[:], lhsT=htT[:], rhs=W2Ts[:],
                                     start=False, stop=True)
                    nc.vector.tensor_reduce(out=mx_all[:, t:t + 1], in_=lg[:],
                                            axis=mybir.AxisListType.X, op=OP.max,
                                            negate=True)
                    scr40 = cpool.tile([128, C], F32, tag="scr40")
                    nc.scalar.activation(scr40[:], lg[:], AF.Exp,
                                         bias=mx_all[:, t:t + 1],
                                         accum_out=se_all[:, t:t + 1])
                    nc.vector.tensor_copy(outs[:, t, :], lg[:])
                nc.scalar.activation(lse_all[:], se_all[:], AF.Ln)
                for t in range(TPC):
                    nc.vector.tensor_scalar(
                        out=outs[:, t, :], in0=outs[:, t, :],
                        scalar1=mx_all[:, t:t + 1], scalar2=lse_all[:, t:t + 1],
                        op0=OP.add, op1=OP.subtract)
                nc.sync.dma_start(out_h[:].rearrange("(t p) c -> p t c", p=128),
                                  outs[:])
    nc.compile()
    return nc


def run(cfg: Cfg, inputs: dict, trace: bool = False):
    in_maps, orders, CACB = host_prep(cfg, **inputs)
    nc = build_nc(cfg, CACB)
    res = bass_utils.run_bass_kernel_spmd(
        nc, in_maps, core_ids=list(range(cfg.M)), trace=trace)
    out = np.empty((cfg.N, cfg.C), dtype=np.float32)
    for k in range(cfg.M):
        out[k * cfg.NSH + orders[k]] = np.asarray(res.results[k]["out"],
                                                  np.float32)[:cfg.NSH]
    return out, res


def kernel(x, edge_index, W1, b1, W2, b2, att_l, att_r):
    cfg = Cfg()
    out, _ = run(cfg, dict(x=np.asarray(x, np.float32),
                           edge_index=np.asarray(edge_index),
                           W1=W1, b1=b1, W2=W2, b2=b2,
                           att_l=att_l, att_r=att_r))
    return out


# revision 18
# speedup vs baseline: 3.6839x; 1.0067x over previous
"""FAGCN (4-layer FAConv + lin1/lin2 + log_softmax) on 8 Trainium2 cores.

6.27ms (baseline) -> 1.71ms. What was changed and why:
- dma_gather desc-gen round-robins over 4 SWDGE queues: the gather ucode
  runs on Q7 core pair (2*q, 2*q+1), so 4 queues give ~4x descriptor
  throughput (baseline was 78% GpSimd-busy on queue 0 alone).
- Slot-padding gather indices are SPREAD across the table window instead of
  0: tens of thousands of concurrent reads of one 256B row serialize at the
  HBM bank and halve the whole drain rate (measured 2.3 -> 4.4 ns/idx with
  25% same-row indices).
- bf16 table rows of 256B (h(64)|al|junk) halve gather + AllGather bytes.
- Self-loop messages computed on-chip (batched per-node DVE) instead of
  gathered; (d, n2-n0) node ordering + optimal per-tile window split lower
  shared CA/CB slot padding 1.48x -> 1.30x.
- Per-slot DVE scaling replaced by per-chunk stride-0-broadcast multiplies;
  al/ar node scores batched via bcast-mult + innermost tensor_reduce.
- Per-slot identity matmuls replaced by 4-slot-wide matmuls into [128,256]
  of a PSUM bank per tile (ragged groups nest in covered columns); blocks
  summed by a batched copy+add fold per 4-tile group.
- Phase 0 (x @ W1) in bf16; h/raw/psum accumulation kept in f32
  (final rel err 9.2e-04 vs 2e-2 budget).
"""
import numpy as np
from dataclasses import dataclass

import concourse.bass as bass
import concourse.bacc as bacc
import concourse.tile as tile
import concourse.mybir as mybir
from concourse import bass_utils
from concourse.masks import make_identity

F32 = mybir.dt.float32
BF16 = mybir.dt.bfloat16
I16 = mybir.dt.int16
AF = mybir.ActivationFunctionType
OP = mybir.AluOpType


@dataclass
class Cfg:
    N: int = 50000
    E: int = 800000
    F: int = 512
    H: int = 64
    C: int = 40
    L: int = 4
    EPS: float = 0.2
    M: int = 8           # cores
    CHUNK_COLS: int = 32
    WINDOW: int = 32768  # dma_gather int16 index limit
    QUEUES: int = 4      # SWDGE queues for gather desc-gen
    GT: int = 4          # tiles per psum fold group

    @property
    def NSH(self):
        return self.N // self.M

    @property
    def TPC(self):
        return (self.NSH + 127) // 128

    @property
    def NSHP(self):
        return self.TPC * 128

    @property
    def RF(self):
        return self.NSHP * self.M


def host_prep(cfg: Cfg, x, edge_index, W1, b1, W2, b2, att_l, att_r):
    """Shard + order nodes, build window-split gather/norm arrays (no loops)."""
    import ml_dtypes
    N, M, NSH, NSHP, TPC = cfg.N, cfg.M, cfg.NSH, cfg.NSHP, cfg.TPC
    src = np.asarray(edge_index[0], dtype=np.int64)
    dst = np.asarray(edge_index[1], dtype=np.int64)
    deg = np.bincount(dst, minlength=N).astype(np.float32) + 1.0  # + self loop
    dinv = (1.0 / np.sqrt(deg)).astype(np.float32)
    norm_e = (dinv[src] * dinv[dst]).astype(np.float32)
    core_of = dst // NSH
    B_BASE = cfg.RF - cfg.WINDOW

    per_core = []  # (es, dl, n0_, n1_)? computed per iteration
    for k in range(M):
        m = core_of == k
        per_core.append((src[m], dst[m] - k * NSH, norm_e[m]))

    def feats(grow_map):
        out = []
        for k in range(M):
            es, ds, _ = per_core[k]
            grow = grow_map[es]
            cls = np.where(grow >= cfg.WINDOW, 2,
                           np.where(grow >= B_BASE, 1, 0))
            n0 = np.bincount(ds[cls == 0], minlength=NSH)
            n2 = np.bincount(ds[cls == 2], minlength=NSH)
            d = np.bincount(ds, minlength=NSH)
            out.append((n0, n2, d))
        return out

    def grow_from(orders):
        gm = np.empty(N, dtype=np.int64)
        for k in range(M):
            inv = np.empty(NSH, dtype=np.int64)
            inv[orders[k]] = np.arange(NSH)
            gm[k * NSH:(k + 1) * NSH] = k * NSHP + inv
        return gm

    orders = [np.arange(NSH) for _ in range(M)]
    for _ in range(2):
        f = feats(grow_from(orders))
        orders = [np.lexsort((f[k][1] - f[k][0], -f[k][2])) for k in range(M)]
    grow_map = grow_from(orders)
    f = feats(grow_map)

    # shared per-tile window budgets
    A0 = np.zeros(TPC, dtype=np.int64)
    B2 = np.zeros(TPC, dtype=np.int64)
    D = np.zeros(TPC, dtype=np.int64)
    inv_orders = []
    for k in range(M):
        inv = np.empty(NSH, dtype=np.int64)
        inv[orders[k]] = np.arange(NSH)
        inv_orders.append(inv)
        n0o = np.zeros(NSHP, dtype=np.int64)
        n2o = np.zeros(NSHP, dtype=np.int64)
        do = np.zeros(NSHP, dtype=np.int64)
        n0o[:NSH] = f[k][0][orders[k]]
        n2o[:NSH] = f[k][1][orders[k]]
        do[:NSH] = f[k][2][orders[k]]
        A0 = np.maximum(A0, n0o.reshape(TPC, 128).max(1))
        B2 = np.maximum(B2, n2o.reshape(TPC, 128).max(1))
        D = np.maximum(D, do.reshape(TPC, 128).max(1))
    cost = np.maximum(A0 + B2, np.maximum(D, 4))
    CA = np.maximum(np.maximum(A0, cost - B2), 4)  # >=1 full 4-slot A group
    CB = np.maximum(cost - CA, B2)
    assert CA.max() <= cfg.CHUNK_COLS and CB.max() <= cfg.CHUNK_COLS, \
        (CA.max(), CB.max())
    offA = np.zeros(TPC + 1, dtype=np.int64)
    np.cumsum(CA, out=offA[1:])
    offB = np.zeros(TPC + 1, dtype=np.int64)
    np.cumsum(CB, out=offB[1:])
    TA, TB = int(offA[-1]), int(offB[-1])

    def wrap16(lst16):
        a = lst16.reshape(-1, 16).T.copy()
        return np.tile(a, (8, 1)).astype(np.int16)

    perm_f = None  # no feature permutation
    in_maps = []
    for k in range(M):
        es, ds, en = per_core[k]
        rk = inv_orders[k][ds]
        grow = grow_map[es]
        cls = np.where(grow >= cfg.WINDOW, 2,
                       np.where(grow >= B_BASE, 1, 0)).astype(np.int8)
        n0 = np.bincount(rk[cls == 0], minlength=NSHP)
        n1 = np.bincount(rk[cls == 1], minlength=NSHP)
        d = np.bincount(rk, minlength=NSHP)
        t_all0 = np.arange(NSHP) // 128
        nlo = np.minimum(CA[t_all0][np.arange(NSHP)], n0 + n1)
        nlo = np.maximum(nlo, n0)
        assert (d - nlo <= CB[t_all0]).all()

        o = np.lexsort((cls, rk))
        rk_s, en_s, grow_s, cls_s = rk[o], en[o], grow[o], cls[o]
        dl = np.bincount(rk_s, minlength=NSHP)
        run0 = np.repeat(np.cumsum(np.concatenate([[0], dl]))[:-1], dl)
        j = np.arange(len(rk_s)) - run0
        is_lo = j < nlo[rk_s]
        p_all = rk_s % 128
        t_all = rk_s // 128
        colA = offA[t_all] + j
        colB = offB[t_all] + (j - nlo[rk_s])
        posA = colA[is_lo] * 128 + p_all[is_lo]
        posB = colB[~is_lo] * 128 + p_all[~is_lo]

        # padding slots get SPREAD indices (not 0): thousands of concurrent
        # reads of one row serialize at the HBM bank and stall the drain.
        idxA = (np.arange(TA * 128, dtype=np.int64) * 2654435761) % cfg.WINDOW
        idxA[posA] = grow_s[is_lo]
        normv = np.zeros((128, TA + TB), dtype=np.float32)
        normv[p_all[is_lo], colA[is_lo]] = en_s[is_lo]
        assert idxA.max() < cfg.WINDOW
        if TB > 0:
            idxB = (np.arange(TB * 128, dtype=np.int64) * 2654435761) % cfg.WINDOW
            idxB[posB] = grow_s[~is_lo] - B_BASE
            normv[p_all[~is_lo], TA + colB[~is_lo]] = en_s[~is_lo]
            assert idxB.min() >= 0 and idxB.max() < cfg.WINDOW

        xk = np.zeros((cfg.F, NSHP), dtype=ml_dtypes.bfloat16)
        xk[:, :NSH] = np.asarray(x[k * NSH:(k + 1) * NSH],
                                 np.float32)[orders[k]].T.astype(
                                     ml_dtypes.bfloat16)
        dinv2 = np.zeros((128, TPC), dtype=np.float32)
        dk = dinv[k * NSH:(k + 1) * NSH][orders[k]] ** 2
        dinv2[:, :] = np.pad(dk, (0, NSHP - NSH)).reshape(TPC, 128).T

        im = {
            "xT": np.ascontiguousarray(xk),
            "W1T": np.ascontiguousarray(
                np.asarray(W1, np.float32).T.astype(ml_dtypes.bfloat16)),
            "b1": np.asarray(b1, np.float32).reshape(1, cfg.H).astype(
                ml_dtypes.bfloat16),
            "W2T": np.ascontiguousarray(np.asarray(W2, np.float32).T),
            "b2": np.asarray(b2, np.float32).reshape(1, cfg.C),
            "attl": np.asarray(att_l, np.float32).reshape(1, -1),
            "attr": np.asarray(att_r, np.float32).reshape(1, -1),
            "idxA": wrap16(idxA.astype(np.int16)),
            "normv": normv.astype(ml_dtypes.bfloat16),
            "dinv2": dinv2,
        }
        if TB > 0:
            im["idxB"] = wrap16(idxB.astype(np.int16))
        in_maps.append(im)
    return in_maps, orders, (CA.tolist(), CB.tolist())


def build_nc(cfg: Cfg, CACB):
    CA, CB = (np.asarray(v, dtype=np.int64) for v in CACB)
    TPC, H, C, L = cfg.TPC, cfg.H, cfg.C, cfg.L
    offA = np.zeros(TPC + 1, dtype=np.int64)
    np.cumsum(CA, out=offA[1:])
    offB = np.zeros(TPC + 1, dtype=np.int64)
    np.cumsum(CB, out=offB[1:])
    TA, TB = int(offA[-1]), int(offB[-1])
    NSLC = cfg.F // 128

    nc = bacc.Bacc("TRN2", target_bir_lowering=False, debug=False,
                   num_devices=cfg.M, num_swdge_queues=cfg.QUEUES)
    xT_h = nc.dram_tensor("xT", [cfg.F, cfg.NSHP], BF16, kind="ExternalInput")
    W1T_h = nc.dram_tensor("W1T", [cfg.F, H], BF16, kind="ExternalInput")
    b1_h = nc.dram_tensor("b1", [1, H], BF16, kind="ExternalInput")
    W2T_h = nc.dram_tensor("W2T", [H, C], F32, kind="ExternalInput")
    b2_h = nc.dram_tensor("b2", [1, C], F32, kind="ExternalInput")
    attl_h = nc.dram_tensor("attl", [1, L * H], F32, kind="ExternalInput")
    attr_h = nc.dram_tensor("attr", [1, L * H], F32, kind="ExternalInput")
    idxA_h = nc.dram_tensor("idxA", [128, 8 * TA], I16, kind="ExternalInput")
    if TB > 0:
        idxB_h = nc.dram_tensor("idxB", [128, 8 * TB], I16, kind="ExternalInput")
    normv_h = nc.dram_tensor("normv", [128, TA + TB], BF16, kind="ExternalInput")
    dinv2_h = nc.dram_tensor("dinv2", [128, TPC], F32, kind="ExternalInput")
    out_h = nc.dram_tensor("out", [cfg.NSHP, C], F32, kind="ExternalOutput")

    # chunks: consecutive tiles with both window spans <= CHUNK_COLS
    chunks = []
    t0 = 0
    for t in range(TPC + 1):
        if t == TPC or (t > t0 and
                        (offA[t] - offA[t0] + CA[t] > cfg.CHUNK_COLS or
                         offB[t] - offB[t0] + CB[t] > cfg.CHUNK_COLS)):
            if t > t0:
                chunks.append((t0, t))
            t0 = t
    chunk_of = {}
    for ci, (a, b) in enumerate(chunks):
        for t in range(a, b):
            chunk_of[t] = ci

    with tile.TileContext(nc) as tc:
        with tc.tile_pool(name="dram", bufs=2, space="DRAM") as dram, \
             tc.tile_pool(name="pers", bufs=1) as pers, \
             tc.tile_pool(name="gpool", bufs=4) as gpool, \
             tc.tile_pool(name="cpool", bufs=4) as cpool, \
             tc.tile_pool(name="mpool", bufs=6) as mpool, \
             tc.tile_pool(name="fpool", bufs=2) as fpool, \
             tc.tile_pool(name="nspool", bufs=1) as nspool:

            onesb = pers.tile([1, 128], BF16)
            nc.vector.memset(onesb[:], 1.0)
            ones = pers.tile([1, 128], F32)
            nc.vector.memset(ones[:], 1.0)
            ident = pers.tile([128, 128], F32)
            make_identity(nc, ident[:])
            identb = pers.tile([128, 128], BF16)
            nc.vector.tensor_copy(identb[:], ident[:])
            b1s = pers.tile([1, H], BF16)
            nc.sync.dma_start(b1s[:], b1_h[:])
            b2s = pers.tile([1, C], F32)
            nc.sync.dma_start(b2s[:], b2_h[:])
            W2Ts = pers.tile([H, C], F32)
            nc.sync.dma_start(W2Ts[:], W2T_h[:])
            W1Ts = pers.tile([128, NSLC, H], BF16)
            nc.sync.dma_start(W1Ts[:], W1T_h[:].rearrange("(s p) h -> p s h", p=128))
            attls = pers.tile([1, L * H], F32)
            nc.sync.dma_start(attls[:], attl_h[:])
            attrs = pers.tile([1, L * H], F32)
            nc.sync.dma_start(attrs[:], attr_h[:])
            idxA = pers.tile([128, 8 * TA], I16)
            nc.sync.dma_start(idxA[:], idxA_h[:])
            if TB > 0:
                idxB = pers.tile([128, 8 * TB], I16)
                nc.sync.dma_start(idxB[:], idxB_h[:])
            normv = pers.tile([128, TA + TB], BF16)
            nc.sync.dma_start(normv[:], normv_h[:])
            dinv2 = pers.tile([128, TPC], F32)
            nc.sync.dma_start(dinv2[:], dinv2_h[:])

            attbc = pers.tile([128, max(2 * L, 1), H], F32)
            with tc.tile_pool(name="bpsum", bufs=2, space="PSUM") as bpsum:
                for l in range(L):
                    for j, srcrow in enumerate((attls, attrs)):
                        bc = bpsum.tile([128, H], F32, tag="bc")
                        nc.tensor.matmul(bc[:], lhsT=ones[:],
                                         rhs=srcrow[0:1, l * H:(l + 1) * H],
                                         start=True, stop=True)
                        nc.vector.tensor_copy(attbc[:, 2 * l + j, :], bc[:])

            stage = pers.tile([128, TPC, H], F32)
            raw = pers.tile([128, TPC, H], F32)
            al_sb = pers.tile([128, TPC], F32)
            ar_sb = pers.tile([128, TPC], F32)
            export = pers.tile([128, TPC, H + 1], BF16)
            outs = pers.tile([128, TPC, C], F32)
            mx_all = pers.tile([128, TPC], F32)
            se_all = pers.tile([128, TPC], F32)
            lse_all = pers.tile([128, TPC], F32)

            def node_scores(l):
                """al/ar for layer l, batched: mult-bcast + innermost reduce."""
                for j, dst in ((0, al_sb), (1, ar_sb)):
                    tmp = nspool.tile([128, TPC, H], F32, tag="nsc")
                    nc.vector.tensor_tensor(
                        out=tmp[:], in0=stage[:],
                        in1=attbc[:, 2 * l + j, :].unsqueeze(1).broadcast_to(
                            (128, TPC, H)), op=OP.mult)
                    nc.vector.tensor_reduce(out=dst[:], in_=tmp[:],
                                            axis=mybir.AxisListType.X,
                                            op=OP.add)

            def do_export():
                nc.vector.tensor_copy(export[:, :, 0:H], stage[:])
                nc.vector.tensor_copy(export[:, :, H:H + 1],
                                      al_sb[:].unsqueeze(2))

            # ---- phase 0: h0 = relu(x @ W1.T + b1)
            XG = 4
            with tc.tile_pool(name="xpool", bufs=2) as xpool, \
                 tc.tile_pool(name="ppsum", bufs=4, space="PSUM") as ppsum:
                for g0 in range(0, TPC, XG):
                    g1 = min(g0 + XG, TPC)
                    cw = (g1 - g0) * 128
                    xt = xpool.tile([128, NSLC, cw], BF16, tag="xt")
                    nc.sync.dma_start(
                        xt[:, :, :],
                        xT_h[:, g0 * 128:g1 * 128].rearrange("(s p) c -> p s c", p=128))
                    for t in range(g0, g1):
                        lc = (t - g0) * 128
                        acc = ppsum.tile([128, H], F32, tag="acc")
                        nc.tensor.matmul(acc[:], lhsT=onesb[:], rhs=b1s[:],
                                         start=True, stop=False)
                        for s in range(NSLC):
                            nc.tensor.matmul(acc[:], lhsT=xt[:, s, lc:lc + 128],
                                             rhs=W1Ts[:, s, :],
                                             start=False, stop=(s == NSLC - 1))
                        nc.scalar.activation(stage[:, t, :], acc[:], AF.Relu)
                        nc.vector.tensor_copy(raw[:, t, :], stage[:, t, :])
            node_scores(0)
            do_export()

            # ---- layers
            qctr = 0
            layer_psum = tc.tile_pool(name="qpsum", bufs=3, space="PSUM")
            qpsum = layer_psum.__enter__()
            for l in range(L):
                tbl_in = dram.tile([cfg.NSHP, 128], BF16, tag="tbl_in")
                tbl_full = dram.tile([cfg.RF, 128], BF16, tag="tbl_full",
                                     addr_space="Shared")
                nc.sync.dma_start(
                    tbl_in[:].rearrange("(t p) e -> p t e", p=128)[:, :, 0:H + 1],
                    export[:])
                nc.gpsimd.collective_compute(
                    "AllGather", OP.bypass,
                    replica_groups=[list(range(cfg.M))],
                    ins=[tbl_in.opt()], outs=[tbl_full.opt()])

                # self-loop coefficient per node
                cs = cpool.tile([128, TPC], F32, tag="cs")
                nc.vector.tensor_tensor(out=cs[:], in0=al_sb[:], in1=ar_sb[:],
                                        op=OP.add)
                nc.scalar.activation(cs[:], cs[:], AF.Tanh)
                nc.vector.tensor_tensor(out=cs[:], in0=cs[:], in1=dinv2[:],
                                        op=OP.mult)

                cur_chunk = -1
                gA = gB = msgA = msgB = None
                po = None
                g_t0 = 0

                def open_chunk(ci):
                    nonlocal gA, gB, msgA, msgB, qctr
                    ct0, ct1 = chunks[ci]
                    cA0, cA1 = int(offA[ct0]), int(offA[ct1])
                    cB0, cB1 = int(offB[ct0]), int(offB[ct1])
                    spanA, spanB = cA1 - cA0, cB1 - cB0
                    gA = gpool.tile([128, cfg.CHUNK_COLS, 128], BF16, tag="gA")
                    nc.gpsimd.dma_gather(
                        out_ap=gA[:, :spanA, :],
                        in_ap=tbl_full[:cfg.WINDOW, :],
                        idxs_ap=idxA[:, 8 * cA0:8 * cA1],
                        num_idxs=128 * spanA, num_idxs_reg=128 * spanA,
                        elem_size=128, single_packet=False,
                        queue_num=qctr % cfg.QUEUES)
                    qctr += 1
                    if TB > 0 and spanB > 0:
                        gB = gpool.tile([128, cfg.CHUNK_COLS, 128], BF16,
                                        tag="gB")
                        nc.gpsimd.dma_gather(
                            out_ap=gB[:, :spanB, :],
                            in_ap=tbl_full[cfg.RF - cfg.WINDOW:, :],
                            idxs_ap=idxB[:, 8 * cB0:8 * cB1],
                            num_idxs=128 * spanB, num_idxs_reg=128 * spanB,
                            elem_size=128, single_packet=False,
                            queue_num=qctr % cfg.QUEUES)
                        qctr += 1
                    # coeff: tanh(al_src + ar_dst) per tile, then * norm batched
                    cfA = cpool.tile([128, cfg.CHUNK_COLS], BF16, tag="cfA")
                    cfB = cpool.tile([128, cfg.CHUNK_COLS], BF16, tag="cfB")
                    for t in range(ct0, ct1):
                        nA = int(CA[t])
                        lcA = int(offA[t]) - cA0
                        nc.scalar.activation(cfA[:, lcA:lcA + nA],
                                             gA[:, lcA:lcA + nA, H],
                                             AF.Tanh, bias=ar_sb[:, t:t + 1])
                        nB = int(CB[t])
                        if nB > 0:
                            lcB = int(offB[t]) - cB0
                            nc.scalar.activation(cfB[:, lcB:lcB + nB],
                                                 gB[:, lcB:lcB + nB, H],
                                                 AF.Tanh, bias=ar_sb[:, t:t + 1])
                    nc.vector.tensor_tensor(
                        out=cfA[:, :spanA], in0=cfA[:, :spanA],
                        in1=normv[:, cA0:cA1], op=OP.mult)
                    msgA = mpool.tile([128, cfg.CHUNK_COLS, H], BF16, tag="mA")
                    nc.vector.tensor_tensor(
                        out=msgA[:, :spanA, :], in0=gA[:, :spanA, 0:H],
                        in1=cfA[:, :spanA].unsqueeze(2).broadcast_to(
                            (128, spanA, H)), op=OP.mult)
                    if spanB > 0:
                        nc.vector.tensor_tensor(
                            out=cfB[:, :spanB], in0=cfB[:, :spanB],
                            in1=normv[:, TA + cB0:TA + cB1], op=OP.mult)
                        msgB = mpool.tile([128, cfg.CHUNK_COLS, H], BF16,
                                          tag="mB")
                        nc.vector.tensor_tensor(
                            out=msgB[:, :spanB, :], in0=gB[:, :spanB, 0:H],
                            in1=cfB[:, :spanB].unsqueeze(2).broadcast_to(
                                (128, spanB, H)), op=OP.mult)

                def fold_group(t0g, t1g):
                    n = t1g - t0g
                    # one PSUM operand per DVE op: copy, then accumulate
                    c0 = fpool.tile([128, cfg.GT, 128], F32, tag="c0")
                    nc.vector.tensor_copy(c0[:, :n, :],
                                          po[:, t0g - g_t0:t1g - g_t0, 0:128])
                    nc.vector.tensor_tensor(out=c0[:, :n, :],
                                            in0=c0[:, :n, :],
                                            in1=po[:, t0g - g_t0:t1g - g_t0, 128:256],
                                            op=OP.add)
                    f3 = fpool.tile([128, cfg.GT, H], F32, tag="f3")
                    nc.vector.tensor_tensor(out=f3[:, :n, :],
                                            in0=c0[:, :n, 0:H],
                                            in1=c0[:, :n, H:128], op=OP.add)
                    ms = fpool.tile([128, cfg.GT, H], F32, tag="ms")
                    nc.vector.tensor_tensor(
                        out=ms[:, :n, :], in0=stage[:, t0g:t1g, :],
                        in1=cs[:, t0g:t1g].unsqueeze(2).broadcast_to(
                            (128, n, H)), op=OP.mult)
                    nc.vector.scalar_tensor_tensor(
                        out=stage[:, t0g:t1g, :], in0=raw[:, t0g:t1g, :],
                        scalar=cfg.EPS, in1=f3[:, :n, :],
                        op0=OP.mult, op1=OP.add)
                    nc.vector.tensor_tensor(out=stage[:, t0g:t1g, :],
                                            in0=stage[:, t0g:t1g, :],
                                            in1=ms[:, :n, :], op=OP.add)

                for t in range(TPC):
                    ci = chunk_of[t]
                    if ci != cur_chunk:
                        open_chunk(ci)
                        cur_chunk = ci
                    if t % cfg.GT == 0:
                        g_t0 = t
                        po = qpsum.tile([128, cfg.GT, 256], F32, tag="po")
                    j = t - g_t0
                    ct0 = chunks[ci][0]
                    lcA = int(offA[t]) - int(offA[ct0])
                    lcB = int(offB[t]) - int(offB[ct0])
                    nA, nB = int(CA[t]), int(CB[t])
                    W = 4  # slots per wide matmul (256 psum cols)
                    ops = []  # (msg, lc, nslots) full groups first, then ragged
                    for g in range(nA // W):
                        ops.append((msgA, lcA + W * g, W))
                    for g in range(nB // W):
                        ops.append((msgB, lcB + W * g, W))
                    if nA % W:
                        ops.append((msgA, lcA + W * (nA // W), nA % W))
                    if nB % W:
                        ops.append((msgB, lcB + W * (nB // W), nB % W))
                    assert ops and ops[0][2] == W
                    for oi, (mm, lc, ns) in enumerate(ops):
                        cols = 64 * ns
                        nc.tensor.matmul(
                            po[:, j, 256 - cols:256],
                            lhsT=identb[:],
                            rhs=mm[:, lc:lc + ns, :],
                            start=(oi == 0), stop=(oi == len(ops) - 1),
                            skip_group_check=True)
                    if t % cfg.GT == cfg.GT - 1 or t == TPC - 1:
                        fold_group(g_t0, t + 1)

                if l < L - 1:
                    node_scores(l + 1)
                    do_export()
            layer_psum.__exit__(None, None, None)

            # ---- logits + log_softmax
            with tc.tile_pool(name="spool", bufs=2) as spool, \
                 tc.tile_pool(name="lpsum", bufs=4, space="PSUM") as lpsum:
                for t in range(TPC):
                    tr = lpsum.tile([H, 128], F32, tag="tr")
                    nc.tensor.transpose(out=tr[:], in_=stage[:, t, :],
                                        identity=ident[:])
                    htT = spool.tile([H, 128], F32, tag="htT")
                    nc.vector.tensor_copy(htT[:], tr[:])
                    lg = lpsum.tile([128, C], F32, tag="lg")
                    nc.tensor.matmul(lg[:], lhsT=ones[:], rhs=b2s[:],
                                     start=True, stop=False)
                    nc.tensor.matmul(lg[:], lhsT=htT[:], rhs=W2Ts[:],
                                     start=False, stop=True)
                    nc.vector.tensor_reduce(out=mx_all[:, t:t + 1], in_=lg[:],
                                            axis=mybir.AxisListType.X, op=OP.max,
                                            negate=True)
                    scr40 = cpool.tile([128, C], F32, tag="scr40")
                    nc.scalar.activation(scr40[:], lg[:], AF.Exp,
                                         bias=mx_all[:, t:t + 1],
                                         accum_out=se_all[:, t:t + 1])
                    nc.vector.tensor_copy(outs[:, t, :], lg[:])
                nc.scalar.activation(lse_all[:], se_all[:], AF.Ln)
                for t in range(TPC):
                    nc.vector.tensor_scalar(
                        out=outs[:, t, :], in0=outs[:, t, :],
                        scalar1=mx_all[:, t:t + 1], scalar2=lse_all[:, t:t + 1],
                        op0=OP.add, op1=OP.subtract)
                nc.sync.dma_start(out_h[:].rearrange("(t p) c -> p t c", p=128),
                                  outs[:])
    nc.compile()
    return nc


def run(cfg: Cfg, inputs: dict, trace: bool = False):
    in_maps, orders, CACB = host_prep(cfg, **inputs)
    nc = build_nc(cfg, CACB)
    res = bass_utils.run_bass_kernel_spmd(
        nc, in_maps, core_ids=list(range(cfg.M)), trace=trace)
    out = np.empty((cfg.N, cfg.C), dtype=np.float32)
    for k in range(cfg.M):
        out[k * cfg.NSH + orders[k]] = np.asarray(res.results[k]["out"],
                                                  np.float32)[:cfg.NSH]
    return out, res


def kernel(x, edge_index, W1, b1, W2, b2, att_l, att_r):
    cfg = Cfg()
    out, _ = run(cfg, dict(x=np.asarray(x, np.float32),
                           edge_index=np.asarray(edge_index),
                           W1=W1, b1=b1, W2=W2, b2=b2,
                           att_l=att_l, att_r=att_r))
    return out
